# revision 1
# baseline (speedup 1.0000x reference)
"""Trainium2 Bass kernel: MHSA with multi-head relative position embedding.

Sharding: data-parallel over batch — 16 batches / 8 cores = 2 batches per core,
each core computes all 8 heads for its 2 batches. No collectives needed.

Math per batch (N=784 tokens, C=512, 8 heads x 64 dim):
  qkv = x @ w_qkv                  (q-columns pre-scaled by 1/8 on host)
  scores_T[k,q] = k_h^T q_h + biasT[h,k,q]   (bias Toeplitz-gathered on host,
                                              static rel_idx, passed transposed)
  E = exp(scores_T)  (no max-subtraction: |scores| < ~2, exp is safe)
  O_T[d,q] = sum_k v_aug[k, d] E[k,q]  with v_aug = [v | 1] -> row 64 = sumexp
  attnout_T = O_T[0:64] / O_T[64]
  out = attnout^T stacked over heads @ w_out

Device layouts: everything transposed (features on partitions) so q^T/k^T are
natural slices of the qkv^T projection; v in token-major from the same x^T via
swapping matmul operands. attn@v is done as (v_aug^T @ E) to avoid transposing
the 784x784 attention matrix; softmax denominator comes free as the ones-column
row of the augmented v.
"""

import numpy as np
import ml_dtypes

B, HH, WW, C = 16, 28, 28, 512
N = HH * WW            # 784 tokens
HEADS, KD = 8, 64
NCORES, BPC = 8, 2     # 8 cores, 2 batches per core
NT, TP = 7, 112        # 784 = 7 tiles of 112 (k / token tiling)
CHUNKS = [(0, 512), (512, 272)]   # q-chunks (PSUM bank = 512 fp32)
CT = 4                 # contraction tiles of 128 over C=512
F_QK_TILES = 8         # 4 q-feature + 4 k-feature tiles of 128

_CACHE = {}


def _rel_index():
    # Faithful to reference._relative_position_index: token r -> (r%28, r//28)
    t = np.arange(N)
    c0, c1 = t % HH, t // HH
    return ((c0[:, None] - c0[None, :] + HH - 1)
            + (c1[:, None] - c1[None, :] + WW - 1) * (2 * HH - 1))  # [q, k]


def build_nc():
    if 'nc' in _CACHE:
        return _CACHE['nc']
    from contextlib import ExitStack
    import concourse.bacc as bacc
    import concourse.mybir as mybir
    import concourse.tile as tile
    from concourse.alu_op_type import AluOpType

    f32 = mybir.dt.float32
    bf16 = mybir.dt.bfloat16
    EXP = mybir.ActivationFunctionType.Exp

    nc = bacc.Bacc("TRN2", debug=False, enable_asserts=False)
    xT_d = nc.dram_tensor("xT", [BPC, C, N], bf16, kind="ExternalInput").ap()
    wqkv_d = nc.dram_tensor("wqkv", [C, 3 * C], bf16, kind="ExternalInput").ap()
    wout_d = nc.dram_tensor("wout", [C, C], bf16, kind="ExternalInput").ap()
    bias_d = nc.dram_tensor("biasT", [HEADS, N, N], bf16, kind="ExternalInput").ap()
    out_d = nc.dram_tensor("out", [BPC, N, C], f32, kind="ExternalOutput").ap()

    with tile.TileContext(nc) as tc, ExitStack() as ctx:
        persist = ctx.enter_context(tc.tile_pool(name="persist", bufs=1))
        xT_pool = ctx.enter_context(tc.tile_pool(name="xTp", bufs=8))
        bias_pool = ctx.enter_context(tc.tile_pool(name="biasp", bufs=21))
        eraw_pool = ctx.enter_context(tc.tile_pool(name="erp", bufs=8))
        e_pool = ctx.enter_context(tc.tile_pool(name="ep", bufs=8))
        r_pool = ctx.enter_context(tc.tile_pool(name="rp", bufs=3))
        osb_pool = ctx.enter_context(tc.tile_pool(name="osbp", bufs=2))
        sc_psum = ctx.enter_context(tc.tile_pool(name="scp", bufs=4, space="PSUM"))
        o_psum = ctx.enter_context(tc.tile_pool(name="opp", bufs=4, space="PSUM"))
        pj_psum = o_psum  # share banks: proj phases and attention never overlap much

        # ---- weights resident in SBUF ----
        wqkv_sb, wout_sb = [], []
        for ci in range(CT):
            w = persist.tile([128, 3 * C], bf16, tag=f"wqkv{ci}")
            nc.sync.dma_start(w, wqkv_d[ci * 128:(ci + 1) * 128, :])
            wqkv_sb.append(w)
        for ci in range(CT):
            w = persist.tile([128, C], bf16, tag=f"wout{ci}")
            nc.sync.dma_start(w, wout_d[ci * 128:(ci + 1) * 128, :])
            wout_sb.append(w)

        qkT, vsb, attnT = {}, {}, {}
        for b in range(BPC):
            for fi in range(CT):
                attnT[b, fi] = persist.tile(
                    [128, N], bf16, tag=f"attnT{b}_{fi}", name=f"attnT{b}_{fi}")

        # ---- qkv projection, split so head-pair 0 can start early ----
        def emit_xt(b):
            tiles = []
            for ci in range(CT):
                xt = xT_pool.tile([128, N], bf16, tag="xT", name=f"xT{b}_{ci}")
                nc.sync.dma_start(xt, xT_d[b, ci * 128:(ci + 1) * 128, :])
                tiles.append(xt)
            return tiles

        def emit_qk_tile(b, ft, xT_sb):
            dst = persist.tile([128, N], bf16, tag=f"qkT{b}_{ft}",
                               name=f"qkT{b}_{ft}")
            qkT[b, ft] = dst
            for (c0w, cw) in CHUNKS:
                ps = pj_psum.tile([128, cw], f32, tag="op", name=f"pj{b}_{ft}_{c0w}")
                for ci in range(CT):
                    nc.tensor.matmul(
                        ps, wqkv_sb[ci][:, ft * 128:(ft + 1) * 128],
                        xT_sb[ci][:, c0w:c0w + cw],
                        start=(ci == 0), stop=(ci == CT - 1))
                nc.vector.tensor_copy(dst[:, c0w:c0w + cw], ps)

        def emit_v(b, xT_sb):
            for t in range(NT):
                vt = persist.tile([TP, HEADS, KD + 1], bf16, tag=f"v{b}_{t}",
                                  name=f"v{b}_{t}")
                vsb[b, t] = vt
                ps = pj_psum.tile([TP, C], f32, tag="op", name=f"pv{b}_{t}")
                for ci in range(CT):
                    nc.tensor.matmul(
                        ps, xT_sb[ci][:, t * TP:(t + 1) * TP],
                        wqkv_sb[ci][:, 2 * C:3 * C],
                        start=(ci == 0), stop=(ci == CT - 1))
                nc.vector.tensor_copy(
                    vt[:, :, 0:KD], ps.rearrange("p (h d) -> p h d", h=HEADS))
                nc.vector.memset(vt[:, :, KD:KD + 1], 1.0)

        # ---- attention for one head pair ----
        def attention_pair(hp):
            h0, h1 = 2 * hp, 2 * hp + 1
            with nc.named_scope(f"headpair{hp}"):
                bias_sb = {}
                for h in (h0, h1):
                    for kt in range(NT):
                        bt = bias_pool.tile([TP, N], bf16, tag="bias",
                                            name=f"bias{h}_{kt}")
                        nc.sync.dma_start(bt, bias_d[h, kt * TP:(kt + 1) * TP, :])
                        bias_sb[h, kt] = bt
                for (c0w, cw) in CHUNKS:
                    # 4 streams: (head even/odd) x (batch) — adjacent even/odd
                    # scores matmuls hit disjoint PE row-groups (base partition
                    # 0 vs 64) and run concurrently; v-matmul trails 1 kt.
                    ops, esbs = {}, {}
                    for h in (h0, h1):
                        for b in range(BPC):
                            ops[h, b] = o_psum.tile(
                                [KD + 1, cw], f32, tag="op",
                                name=f"op{h}_{c0w}_{b}")
                    for kt in range(NT):
                        for b in range(BPC):
                            for h in (h0, h1):
                                r0 = (h % 2) * 64
                                kT_tile = qkT[b, 4 + h // 2]
                                qT_tile = qkT[b, h // 2]
                                scp = sc_psum.tile([TP, cw], f32, tag="sc",
                                                   name=f"sc{h}_{c0w}_{b}_{kt}")
                                nc.tensor.matmul(
                                    scp,
                                    kT_tile[r0:r0 + 64, kt * TP:(kt + 1) * TP],
                                    qT_tile[r0:r0 + 64, c0w:c0w + cw],
                                    start=True, stop=True)
                                eraw = eraw_pool.tile(
                                    [TP, cw], bf16, tag="eraw",
                                    name=f"er{h}_{c0w}_{b}_{kt}")
                                nc.scalar.activation(eraw, scp, EXP)
                                esb = e_pool.tile([TP, cw], bf16, tag="e",
                                                  name=f"e{h}_{c0w}_{b}_{kt}")
                                nc.vector.tensor_tensor(
                                    esb, eraw, bias_sb[h, kt][:, c0w:c0w + cw],
                                    AluOpType.mult)
                                esbs[h, b, kt] = esb
                        if kt >= 1:
                            for b in range(BPC):
                                for h in (h0, h1):
                                    nc.tensor.matmul(
                                        ops[h, b], vsb[b, kt - 1][:, h:h + 1, :],
                                        esbs[h, b, kt - 1],
                                        start=(kt == 1), stop=False)
                    for b in range(BPC):
                        for h in (h0, h1):
                            nc.tensor.matmul(
                                ops[h, b], vsb[b, NT - 1][:, h:h + 1, :],
                                esbs[h, b, NT - 1], start=False, stop=True)
                    for b in range(BPC):
                        for h in (h0, h1):
                            r0 = (h % 2) * 64
                            # normalize: rows 0..63 * (1 / row 64)
                            # (approx-recip must not read PSUM directly: its
                            # bitwise seed sees raw PSUM bits on HW -> garbage)
                            srow = r_pool.tile([1, 512], f32, tag="srow")
                            nc.vector.tensor_copy(
                                srow[:, 0:cw], ops[h, b][KD:KD + 1, 0:cw])
                            rrow = r_pool.tile([1, 512], f32, tag="rrow")
                            nc.vector.reciprocal_approx_fast(
                                rrow[:, 0:cw], srow[:, 0:cw])
                            rb = r_pool.tile([64, 512], f32, tag="rb")
                            nc.gpsimd.partition_broadcast(
                                rb[:, 0:cw], rrow[:, 0:cw])
                            nc.vector.tensor_tensor(
                                attnT[b, h // 2][r0:r0 + 64, c0w:c0w + cw],
                                ops[h, b][0:KD, 0:cw], rb[:, 0:cw],
                                AluOpType.mult)

        # phase 1: minimal inputs for head-pair 0 (q-tile 0, k-tile 4, v)
        xts = {}
        for b in range(BPC):
            with nc.named_scope(f"qkv_early_b{b}"):
                xts[b] = emit_xt(b)
                emit_qk_tile(b, 0, xts[b])
                emit_qk_tile(b, 4, xts[b])
        for b in range(BPC):
            with nc.named_scope(f"v_b{b}"):
                emit_v(b, xts[b])
        # phase 2: pair-0 attention starts while the rest of qkv is emitted
        attention_pair(0)
        # phase 3: remaining q/k feature tiles (ACT is busy with pair 0 here)
        for b in range(BPC):
            with nc.named_scope(f"qkv_rest_b{b}"):
                for ft in (1, 5, 2, 6, 3, 7):
                    emit_qk_tile(b, ft, xts[b])
        # phase 4: remaining head pairs
        for hp in range(1, HEADS // 2):
            attention_pair(hp)

        # ---- output projection ----
        for b in range(BPC):
            with nc.named_scope(f"proj_b{b}"):
                for t in range(NT):
                    ps = pj_psum.tile([TP, C], f32, tag="op")
                    for fi in range(CT):
                        nc.tensor.matmul(
                            ps, attnT[b, fi][:, t * TP:(t + 1) * TP], wout_sb[fi],
                            start=(fi == 0), stop=(fi == CT - 1))
                    osb = osb_pool.tile([TP, C], f32, tag="osb")
                    nc.vector.tensor_copy(osb, ps)
                    nc.sync.dma_start(out_d[b, t * TP:(t + 1) * TP, :], osb)

    nc.compile()
    _CACHE['nc'] = nc
    return nc


def host_prep(x, w_qkv, pos_table, w_out):
    x = np.asarray(x, np.float32).reshape(B, N, C)
    wq = np.array(np.asarray(w_qkv, np.float32), copy=True)
    wq[:, :C] *= np.float32(1.0 / np.sqrt(KD))
    wq_bf = wq.astype(ml_dtypes.bfloat16)
    idx = _rel_index()
    biasT = np.ascontiguousarray(np.exp(
        np.asarray(pos_table, np.float32)[:, idx].transpose(0, 2, 1)
    )).astype(ml_dtypes.bfloat16)
    wout = np.ascontiguousarray(np.asarray(w_out, np.float32)).astype(
        ml_dtypes.bfloat16)
    in_maps = []
    for c in range(NCORES):
        xT = np.ascontiguousarray(
            x[c * BPC:(c + 1) * BPC].transpose(0, 2, 1)).astype(
                ml_dtypes.bfloat16)  # [2, 512, 784]
        in_maps.append({"xT": xT, "wqkv": wq_bf, "wout": wout, "biasT": biasT})
    return in_maps


def run(in_maps, trace=False, trace_cores=None):
    import concourse.bass_utils as bass_utils
    nc = build_nc()
    return bass_utils.run_bass_kernel_spmd(
        nc, in_maps, core_ids=list(range(NCORES)),
        trace=trace, trace_cores=trace_cores)


def kernel(x, w_qkv, pos_table, w_out):
    in_maps = host_prep(x, w_qkv, pos_table, w_out)
    res = run(in_maps)
    out = np.stack([r["out"] for r in res.results])    # [8, 2, 784, 512]
    return np.ascontiguousarray(out.reshape(B, HH, WW, C)).astype(np.float32)



# revision 6
# speedup vs baseline: 1.2641x; 1.2641x over previous
"""Trainium2 Bass kernel V2: MHSA with multi-head relative position embedding.

Data-parallel over batch: 16 batches / 8 cores = 2 per core, all 8 heads local.

V2 vs V1: the TRN2 PE clock p-states (0.65/1.2/2.4 GHz, full speed only after
3us of continuous execution) mean PE idle gaps are doubly expensive. V2 keeps
one continuous PE stream: attention runs one head at a time (7 k-tiles of 112)
with the qkv projections for the NEXT head-pair and the out-projection of the
finished batch interleaved as filler work between attention matmuls. PSUM:
2x scores tiles [112,784] (2 banks each), 1x attn@v accumulator [65,784]
(2 banks), 2x 1-bank filler tiles = 8 banks.

Softmax path per (head, k-tile): scores -> PSUM [112,784] (two matmuls, one
per bank) -> ACT exp (one merged 784-wide instr) -> DVE mult by host-side
exp(bias) Toeplitz gather -> attn@v accumulate with v_aug = [v | 1] so row 64
is sumexp. Normalize: ACT copies sumexp row out, DVE approx-recip, GPSIMD
partition-broadcast, DVE mult -> attnT bf16.
"""

import numpy as np
import ml_dtypes

B, HH, WW, C = 16, 28, 28, 512
N = HH * WW             # 784 tokens
HEADS, KD = 8, 64
NCORES, BPC = 8, 2      # 8 cores, 2 batches per core
NT, TP = 7, 112         # 784 = 7 k/token tiles of 112
CHUNKS = [(0, 512), (512, 272)]   # PSUM bank split of the 784-wide free dim
CT = 4                  # contraction tiles of 128 over C=512
NPAIRS = 4

_CACHE = {}


def _rel_index():
    t = np.arange(N)
    c0, c1 = t % HH, t // HH
    return ((c0[:, None] - c0[None, :] + HH - 1)
            + (c1[:, None] - c1[None, :] + WW - 1) * (2 * HH - 1))  # [q, k]


def build_nc():
    if 'nc' in _CACHE:
        return _CACHE['nc']
    from contextlib import ExitStack
    import concourse.bacc as bacc
    import concourse.mybir as mybir
    import concourse.tile as tile
    from concourse.alu_op_type import AluOpType

    f32 = mybir.dt.float32
    bf16 = mybir.dt.bfloat16
    EXP = mybir.ActivationFunctionType.Exp

    nc = bacc.Bacc("TRN2", debug=False, enable_asserts=False)
    xT_d = nc.dram_tensor("xT", [BPC, C, N], bf16, kind="ExternalInput").ap()
    wqkv_d = nc.dram_tensor("wqkv", [C, 3 * C], bf16, kind="ExternalInput").ap()
    wout_d = nc.dram_tensor("wout", [C, C], bf16, kind="ExternalInput").ap()
    bias_d = nc.dram_tensor("biasT", [HEADS, N, N], bf16, kind="ExternalInput").ap()
    out_d = nc.dram_tensor("out", [BPC, N, C], f32, kind="ExternalOutput").ap()

    with tile.TileContext(nc) as tc, ExitStack() as ctx:
        persist = ctx.enter_context(tc.tile_pool(name="persist", bufs=1))
        bias_pool = ctx.enter_context(tc.tile_pool(name="biasp", bufs=28))
        eraw_pool = ctx.enter_context(tc.tile_pool(name="erp", bufs=3))
        esb_pool = ctx.enter_context(tc.tile_pool(name="ep", bufs=4))
        oc_pool = ctx.enter_context(tc.tile_pool(name="ocp", bufs=2))
        r_pool = ctx.enter_context(tc.tile_pool(name="rp", bufs=3))
        osb_pool = ctx.enter_context(tc.tile_pool(name="osbp", bufs=2))
        sc_psum = ctx.enter_context(tc.tile_pool(name="scp", bufs=2, space="PSUM"))
        op_psum = ctx.enter_context(tc.tile_pool(name="opp", bufs=1, space="PSUM"))
        fl_psum = ctx.enter_context(tc.tile_pool(name="flp", bufs=2, space="PSUM"))

        # ---- persistent SBUF tensors ----
        wqkv_sb = [persist.tile([128, 3 * C], bf16, tag=f"wqkv{ci}", name=f"wqkv{ci}")
                   for ci in range(CT)]
        wout_sb = [persist.tile([128, C], bf16, tag=f"wout{ci}", name=f"woutw{ci}")
                   for ci in range(CT)]
        xts = {(b, ci): persist.tile([128, N], bf16, tag=f"xT{b}_{ci}", name=f"xT{b}_{ci}")
               for b in range(BPC) for ci in range(CT)}
        qkT = {(b, ft): persist.tile([128, N], bf16, tag=f"qkT{b}_{ft}",
                                     name=f"qkT{b}_{ft}")
               for b in range(BPC) for ft in range(8)}
        vsb = {(b, t): persist.tile([TP, HEADS, KD + 1], bf16, tag=f"v{b}_{t}",
                                    name=f"v{b}_{t}")
               for b in range(BPC) for t in range(NT)}
        attnT = {(b, fi): persist.tile([128, N], bf16, tag=f"attnT{b}_{fi}",
                                       name=f"attnT{b}_{fi}")
                 for b in range(BPC) for fi in range(CT)}

        # ---- input DMAs, ordered so the first projections can start early ----
        for ci in range(CT):
            nc.sync.dma_start(xts[0, ci], xT_d[0, ci * 128:(ci + 1) * 128, :])
            nc.sync.dma_start(wqkv_sb[ci], wqkv_d[ci * 128:(ci + 1) * 128, :])
        for ci in range(CT):
            nc.sync.dma_start(xts[1, ci], xT_d[1, ci * 128:(ci + 1) * 128, :])

        bias_sb = {}

        def fetch_bias(pair):
            for h in (2 * pair, 2 * pair + 1):
                for kt in range(NT):
                    bt = bias_pool.tile([TP, N], bf16, tag="bias",
                                        name=f"bias{h}_{kt}")
                    nc.sync.dma_start(bt, bias_d[h, kt * TP:(kt + 1) * TP, :])
                    bias_sb[h, kt] = bt

        fetch_bias(0)
        for ci in range(CT):
            nc.sync.dma_start(wout_sb[ci], wout_d[ci * 128:(ci + 1) * 128, :])
        fetch_bias(1)

        # ---- filler work units (each emits one small PE burst + its copy) ----
        def emit_qk_chunk(b, ft, c0w, cw):
            ps = fl_psum.tile([128, 512], f32, tag="fl", name=f"pj{b}_{ft}_{c0w}")
            for ci in range(CT):
                nc.tensor.matmul(
                    ps[:, 0:cw], wqkv_sb[ci][:, ft * 128:(ft + 1) * 128],
                    xts[b, ci][:, c0w:c0w + cw],
                    start=(ci == 0), stop=(ci == CT - 1))
            nc.vector.tensor_copy(qkT[b, ft][:, c0w:c0w + cw], ps[:, 0:cw])

        def emit_v_tile(b, t, pair):
            ps = fl_psum.tile([128, 512], f32, tag="fl", name=f"pv{b}_{t}_{pair}")
            f0 = 2 * C + 128 * pair
            for ci in range(CT):
                nc.tensor.matmul(
                    ps[0:TP, 0:128], xts[b, ci][:, t * TP:(t + 1) * TP],
                    wqkv_sb[ci][:, f0:f0 + 128],
                    start=(ci == 0), stop=(ci == CT - 1))
            nc.vector.tensor_copy(
                vsb[b, t][:, 2 * pair:2 * pair + 2, 0:KD],
                ps[0:TP, 0:128].rearrange("p (h d) -> p h d", h=2))
            if pair == 0:
                nc.vector.memset(vsb[b, t][:, :, KD:KD + 1], 1.0)

        def emit_oproj_tile(b, t, on_act):
            ps = fl_psum.tile([128, 512], f32, tag="fl", name=f"po{b}_{t}")
            for fi in range(CT):
                nc.tensor.matmul(
                    ps[0:TP, :], attnT[b, fi][:, t * TP:(t + 1) * TP], wout_sb[fi],
                    start=(fi == 0), stop=(fi == CT - 1))
            osb = osb_pool.tile([TP, C], f32, tag="osb", name="osb")
            if on_act:
                nc.scalar.copy(osb, ps[0:TP, :])
            else:
                nc.vector.tensor_copy(osb, ps[0:TP, :])
            nc.sync.dma_start(out_d[b, t * TP:(t + 1) * TP, :], osb)

        def qkv_fillers(pair):
            # projection work needed before `pair` runs: q-tile, k-tile, v
            work = []
            for b in range(BPC):
                for ft in (pair, 4 + pair):
                    for (c0w, cw) in CHUNKS:
                        work.append(lambda b=b, ft=ft, c0w=c0w, cw=cw:
                                    emit_qk_chunk(b, ft, c0w, cw))
            for b in range(BPC):
                for t in range(NT):
                    work.append(lambda b=b, t=t, pair=pair:
                                emit_v_tile(b, t, pair))
            return work

        # ---- attention for one head (pipelined over 7 k-tiles) ----
        def attention_head(pair, b, h, fillers):
            hh = h % 2
            r0 = hh * 64
            qT_tile, kT_tile = qkT[b, pair], qkT[b, 4 + pair]
            op = op_psum.tile([KD + 1, N], f32, tag="op", name=f"op{b}_{h}")
            esbs = {}
            with nc.named_scope(f"attn_p{pair}_b{b}_h{hh}"):
                for kt in range(NT):
                    sc = sc_psum.tile([TP, N], f32, tag="sc",
                                      name=f"sc{b}_{h}_{kt}")
                    for (c0w, cw) in CHUNKS:
                        nc.tensor.matmul(
                            sc[:, c0w:c0w + cw],
                            kT_tile[r0:r0 + 64, kt * TP:(kt + 1) * TP],
                            qT_tile[r0:r0 + 64, c0w:c0w + cw],
                            start=True, stop=True)
                    eraw = eraw_pool.tile([TP, N], bf16, tag="eraw",
                                          name=f"er{b}_{h}_{kt}")
                    nc.scalar.activation(eraw, sc, EXP)
                    esb = esb_pool.tile([TP, N], bf16, tag="e",
                                        name=f"e{b}_{h}_{kt}")
                    nc.vector.tensor_tensor(esb, eraw, bias_sb[h, kt],
                                            AluOpType.mult)
                    esbs[kt] = esb
                    if kt >= 1:
                        for (c0w, cw) in CHUNKS:
                            nc.tensor.matmul(
                                op[:, c0w:c0w + cw], vsb[b, kt - 1][:, h:h + 1, :],
                                esbs[kt - 1][:, c0w:c0w + cw],
                                start=(kt == 1), stop=False)
                    if fillers:
                        fillers.pop(0)()
                for (c0w, cw) in CHUNKS:
                    nc.tensor.matmul(
                        op[:, c0w:c0w + cw], vsb[b, NT - 1][:, h:h + 1, :],
                        esbs[NT - 1][:, c0w:c0w + cw],
                        start=False, stop=True)
                # normalize: two ACT copies free the op psum fast; recip
                # input must live at partition 0 (custom-DVE op) and the
                # value rows copy with an even partition count
                oc = oc_pool.tile([KD, N], f32, tag="oc", name=f"oc{b}_{h}")
                nc.scalar.copy(oc, op[0:KD, :])
                srow = r_pool.tile([1, N], f32, tag="srow", name="srow")
                nc.scalar.copy(srow, op[KD:KD + 1, :])
                rrow = r_pool.tile([1, N], f32, tag="rrow", name="rrow")
                nc.vector.reciprocal_approx_fast(rrow, srow)
                rb = r_pool.tile([64, N], f32, tag="rb", name="rb")
                nc.gpsimd.partition_broadcast(rb, rrow)
                nc.vector.tensor_tensor(
                    attnT[b, pair][r0:r0 + 64, :], oc, rb,
                    AluOpType.mult)

        # ---- schedule ----
        # warmup: projections needed by pair 0
        with nc.named_scope("warmup"):
            for w in qkv_fillers(0):
                w()

        for pair in range(NPAIRS):
            if pair + 2 < NPAIRS:
                fetch_bias(pair + 2)
            fillers = qkv_fillers(pair + 1) if pair + 1 < NPAIRS else []
            if pair == NPAIRS - 1:
                # out-projection of batch 0 fills batch 1's last-pair slack
                fillers_b1 = [lambda t=t: emit_oproj_tile(0, t, on_act=(t % 2 == 0))
                              for t in range(NT)]
            nsteps = 4  # (b, hh) steps per pair
            per = (len(fillers) + nsteps - 1) // nsteps if fillers else 0
            for b in range(BPC):
                for hh in range(2):
                    if pair == NPAIRS - 1 and b == 1:
                        take = fillers_b1[:4]
                        fillers_b1 = fillers_b1[4:]
                    else:
                        take = fillers[:per]
                        fillers = fillers[per:]
                    attention_head(pair, b, 2 * pair + hh, take)
            assert not fillers

        with nc.named_scope("oproj_tail"):
            for t in range(NT):
                if fillers_b1:
                    fillers_b1.pop(0)()
            for t in range(NT):
                emit_oproj_tile(1, t, on_act=(t % 2 == 0))

    nc.compile()
    _CACHE['nc'] = nc
    return nc


def host_prep(x, w_qkv, pos_table, w_out):
    x = np.asarray(x, np.float32).reshape(B, N, C)
    wq = np.array(np.asarray(w_qkv, np.float32), copy=True)
    wq[:, :C] *= np.float32(1.0 / np.sqrt(KD))
    wq_bf = wq.astype(ml_dtypes.bfloat16)
    idx = _rel_index()
    biasT = np.ascontiguousarray(np.exp(
        np.asarray(pos_table, np.float32)[:, idx].transpose(0, 2, 1)
    )).astype(ml_dtypes.bfloat16)
    wout = np.ascontiguousarray(np.asarray(w_out, np.float32)).astype(
        ml_dtypes.bfloat16)
    in_maps = []
    for c in range(NCORES):
        xT = np.ascontiguousarray(
            x[c * BPC:(c + 1) * BPC].transpose(0, 2, 1)).astype(
                ml_dtypes.bfloat16)  # [2, 512, 784]
        in_maps.append({"xT": xT, "wqkv": wq_bf, "wout": wout, "biasT": biasT})
    return in_maps


def run(in_maps, trace=False, trace_cores=None):
    import concourse.bass_utils as bass_utils
    nc = build_nc()
    return bass_utils.run_bass_kernel_spmd(
        nc, in_maps, core_ids=list(range(NCORES)),
        trace=trace, trace_cores=trace_cores)


def kernel(x, w_qkv, pos_table, w_out):
    in_maps = host_prep(x, w_qkv, pos_table, w_out)
    res = run(in_maps)
    out = np.stack([r["out"] for r in res.results])    # [8, 2, 784, 512]
    return np.ascontiguousarray(out.reshape(B, HH, WW, C)).astype(np.float32)


# revision 8
# speedup vs baseline: 1.3439x; 1.0632x over previous
"""Trainium2 Bass kernel V3: MHSA with multi-head relative position embedding.

Data-parallel over batch: 16 batches / 8 cores = 2 per core, all 8 heads local.

V3: one flat software pipeline over 112 (head, k-tile) slots so the PE stream
never breaks (TRN2 p-states make every PE idle gap cost ~3us of half-clock).
Per slot: scores matmuls for slot j, attn@v matmuls for slot j-LAG (the trail
crosses head boundaries), plus column-budget-paced filler work (qkv projection
tiles for future pairs, out-projection once a batch finishes). The attn@v
accumulator is freed by ONE full-tile ACT copy (same ACT cost as copying just
the sumexp row, since ACT time only depends on the free dim), so op needs a
single 2-bank PSUM buffer and the normalize chain (DVE approx-recip, GPSIMD
partition-broadcast, DVE mult) runs entirely off PSUM/critical path.

PSUM: 2x scores [112,784] (2 banks each) + 1x op [65,784] (2 banks) +
2x 1-bank filler = 8 banks.
"""

import numpy as np
import ml_dtypes

B, HH, WW, C = 16, 28, 28, 512
N = HH * WW             # 784 tokens
HEADS, KD = 8, 64
NCORES, BPC = 8, 2      # 8 cores, 2 batches per core
NT, TP = 7, 112         # 784 = 7 k/token tiles of 112
CHUNKS = [(0, 512), (512, 272)]   # PSUM bank split of the 784-wide free dim
CT = 4                  # contraction tiles of 128 over C=512
NPAIRS = 4
LAG = 3                 # attn@v trails scores by this many slots

_CACHE = {}


def _rel_index():
    t = np.arange(N)
    c0, c1 = t % HH, t // HH
    return ((c0[:, None] - c0[None, :] + HH - 1)
            + (c1[:, None] - c1[None, :] + WW - 1) * (2 * HH - 1))  # [q, k]


def build_nc():
    if 'nc' in _CACHE:
        return _CACHE['nc']
    from contextlib import ExitStack
    import concourse.bacc as bacc
    import concourse.mybir as mybir
    import concourse.tile as tile
    from concourse.alu_op_type import AluOpType

    f32 = mybir.dt.float32
    bf16 = mybir.dt.bfloat16
    EXP = mybir.ActivationFunctionType.Exp

    nc = bacc.Bacc("TRN2", debug=False, enable_asserts=False)
    xT_d = nc.dram_tensor("xT", [BPC, C, N], bf16, kind="ExternalInput").ap()
    wqkv_d = nc.dram_tensor("wqkv", [C, 3 * C], bf16, kind="ExternalInput").ap()
    wout_d = nc.dram_tensor("wout", [C, C], bf16, kind="ExternalInput").ap()
    bias_d = nc.dram_tensor("biasT", [HEADS, N, N], bf16, kind="ExternalInput").ap()
    out_d = nc.dram_tensor("out", [BPC, N, C], f32, kind="ExternalOutput").ap()

    with tile.TileContext(nc) as tc, ExitStack() as ctx:
        persist = ctx.enter_context(tc.tile_pool(name="persist", bufs=1))
        bias_pool = ctx.enter_context(tc.tile_pool(name="biasp", bufs=28))
        eraw_pool = ctx.enter_context(tc.tile_pool(name="erp", bufs=3))
        esb_pool = ctx.enter_context(tc.tile_pool(name="ep", bufs=6))
        oc_pool = ctx.enter_context(tc.tile_pool(name="ocp", bufs=3))
        r_pool = ctx.enter_context(tc.tile_pool(name="rp", bufs=4))
        osb_pool = ctx.enter_context(tc.tile_pool(name="osbp", bufs=2))
        sc_psum = ctx.enter_context(tc.tile_pool(name="scp", bufs=2, space="PSUM"))
        op_psum = ctx.enter_context(tc.tile_pool(name="opp", bufs=1, space="PSUM"))
        fl_psum = ctx.enter_context(tc.tile_pool(name="flp", bufs=2, space="PSUM"))

        # ---- persistent SBUF tensors ----
        wqkv_sb = [persist.tile([128, 3 * C], bf16, tag=f"wqkv{ci}",
                                name=f"wqkv{ci}") for ci in range(CT)]
        wout_sb = [persist.tile([128, C], bf16, tag=f"wout{ci}",
                                name=f"woutw{ci}") for ci in range(CT)]
        xts = {(b, ci): persist.tile([128, N], bf16, tag=f"xT{b}_{ci}",
                                     name=f"xT{b}_{ci}")
               for b in range(BPC) for ci in range(CT)}
        qkT = {(b, ft): persist.tile([128, N], bf16, tag=f"qkT{b}_{ft}",
                                     name=f"qkT{b}_{ft}")
               for b in range(BPC) for ft in range(8)}
        vsb = {(b, t): persist.tile([TP, HEADS, KD + 1], bf16, tag=f"v{b}_{t}",
                                    name=f"v{b}_{t}")
               for b in range(BPC) for t in range(NT)}
        attnT = {(b, fi): persist.tile([128, N], bf16, tag=f"attnT{b}_{fi}",
                                       name=f"attnT{b}_{fi}")
                 for b in range(BPC) for fi in range(CT)}

        # ---- input DMAs on two issue queues (sync + gpsimd) so the DGE
        # config time (~600ns per dma_start) does not serialize the start ----
        for ci in range(CT):
            nc.sync.dma_start(xts[0, ci], xT_d[0, ci * 128:(ci + 1) * 128, :])
            nc.sync.dma_start(wqkv_sb[ci], wqkv_d[ci * 128:(ci + 1) * 128, :])
        for ci in range(CT):
            nc.sync.dma_start(xts[1, ci], xT_d[1, ci * 128:(ci + 1) * 128, :])
            nc.sync.dma_start(wout_sb[ci], wout_d[ci * 128:(ci + 1) * 128, :])

        bias_sb = {}

        def fetch_bias_tile(h, kt, queue):
            bt = bias_pool.tile([TP, N], bf16, tag="bias", name=f"bias{h}_{kt}")
            queue.dma_start(bt, bias_d[h, kt * TP:(kt + 1) * TP, :])
            bias_sb[h, kt] = bt

        def fetch_bias(pair, queue):
            for h in (2 * pair, 2 * pair + 1):
                for kt in range(NT):
                    fetch_bias_tile(h, kt, queue)

        fetch_bias(0, nc.sync)
        fetch_bias(1, nc.sync)

        # ---- filler work units; each returns its PE column cost ----
        def emit_qk_chunk(b, ft, c0w, cw):
            ps = fl_psum.tile([128, 512], f32, tag="fl", name=f"pj{b}_{ft}_{c0w}")
            for ci in range(CT):
                nc.tensor.matmul(
                    ps[:, 0:cw], wqkv_sb[ci][:, ft * 128:(ft + 1) * 128],
                    xts[b, ci][:, c0w:c0w + cw],
                    start=(ci == 0), stop=(ci == CT - 1))
            nc.vector.tensor_copy(qkT[b, ft][:, c0w:c0w + cw], ps[:, 0:cw])
            return CT * cw

        def emit_v_tile(b, t, pair):
            ps = fl_psum.tile([128, 512], f32, tag="fl", name=f"pv{b}_{t}_{pair}")
            f0 = 2 * C + 128 * pair
            for ci in range(CT):
                nc.tensor.matmul(
                    ps[0:TP, 0:128], xts[b, ci][:, t * TP:(t + 1) * TP],
                    wqkv_sb[ci][:, f0:f0 + 128],
                    start=(ci == 0), stop=(ci == CT - 1))
            nc.vector.tensor_copy(
                vsb[b, t][:, 2 * pair:2 * pair + 2, 0:KD],
                ps[0:TP, 0:128].rearrange("p (h d) -> p h d", h=2))
            if pair == 0:
                nc.vector.memset(vsb[b, t][:, :, KD:KD + 1], 1.0)
            return CT * 128

        def emit_oproj_prefix(b, t):
            ps = fl_psum.tile([128, 512], f32, tag="fl", name=f"po{b}_{t}")
            for fi in range(CT - 1):
                nc.tensor.matmul(
                    ps[0:TP, :], attnT[b, fi][:, t * TP:(t + 1) * TP], wout_sb[fi],
                    start=(fi == 0), stop=False)
            return ps

        def emit_oproj_finish(b, t, ps, on_act, split_dma=False):
            fi = CT - 1
            nc.tensor.matmul(
                ps[0:TP, :], attnT[b, fi][:, t * TP:(t + 1) * TP], wout_sb[fi],
                start=False, stop=True)
            osb = osb_pool.tile([TP, C], f32, tag="osb", name="osb")
            eng = nc.scalar.copy if on_act else nc.vector.tensor_copy
            if split_dma:
                eng(osb[:, 0:C // 2], ps[0:TP, 0:C // 2])
                nc.sync.dma_start(out_d[b, t * TP:(t + 1) * TP, 0:C // 2],
                                  osb[:, 0:C // 2])
                eng(osb[:, C // 2:C], ps[0:TP, C // 2:C])
                nc.sync.dma_start(out_d[b, t * TP:(t + 1) * TP, C // 2:C],
                                  osb[:, C // 2:C])
            else:
                eng(osb, ps[0:TP, :])
                nc.sync.dma_start(out_d[b, t * TP:(t + 1) * TP, :], osb)

        def emit_oproj_tile(b, t, on_act):
            ps = emit_oproj_prefix(b, t)
            emit_oproj_finish(b, t, ps, on_act)
            return CT * C

        # ---- flat slot pipeline ----
        slots = [(pair, b, hh, kt)
                 for pair in range(NPAIRS)
                 for b in range(BPC)
                 for hh in range(2)
                 for kt in range(NT)]
        nslots = len(slots)

        esbs = {}        # (head-id, kt) -> esb tile
        ops = {}         # head-id -> op psum tile

        def emit_scores(pair, b, h, kt):
            hh = h % 2
            r0 = hh * 64
            sc = sc_psum.tile([TP, N], f32, tag="sc", name=f"sc{b}_{h}_{kt}")
            for (c0w, cw) in CHUNKS:
                nc.tensor.matmul(
                    sc[:, c0w:c0w + cw],
                    qkT[b, 4 + pair][r0:r0 + 64, kt * TP:(kt + 1) * TP],
                    qkT[b, pair][r0:r0 + 64, c0w:c0w + cw],
                    start=True, stop=True)
            eraw = eraw_pool.tile([TP, N], bf16, tag="eraw",
                                  name=f"er{b}_{h}_{kt}")
            nc.scalar.activation(eraw, sc, EXP)
            esb = esb_pool.tile([TP, N], bf16, tag="e", name=f"e{b}_{h}_{kt}")
            nc.vector.tensor_tensor(esb, eraw, bias_sb[h, kt], AluOpType.mult)
            esbs[(pair, b, h), kt] = esb

        def emit_attnv(pair, b, h, kt):
            hid = (pair, b, h)
            if kt == 0:
                ops[hid] = op_psum.tile([KD + 1, N], f32, tag="op",
                                        name=f"op{b}_{h}")
            op = ops[hid]
            for (c0w, cw) in CHUNKS:
                nc.tensor.matmul(
                    op[:, c0w:c0w + cw], vsb[b, kt][:, h:h + 1, :],
                    esbs[hid, kt][:, c0w:c0w + cw],
                    start=(kt == 0), stop=(kt == NT - 1))

        def emit_normalize(pair, b, h):
            hh = h % 2
            r0 = hh * 64
            op = ops.pop((pair, b, h))
            # two ACT copies free the op psum fast; the custom-DVE recip needs
            # its input at partition 0 and the value rows copy with an even
            # partition count (the fused [65,784] variant corrupts on HW)
            oc = oc_pool.tile([KD, N], f32, tag="oc", name=f"oc{b}_{h}")
            nc.scalar.copy(oc, op[0:KD, :])
            srow = r_pool.tile([1, N], f32, tag="srow", name="srow")
            nc.scalar.copy(srow, op[KD:KD + 1, :])
            rrow = r_pool.tile([1, N], f32, tag="rrow", name="rrow")
            nc.vector.reciprocal_approx_fast(rrow, srow)
            rb = r_pool.tile([64, N], f32, tag="rb", name="rb")
            nc.gpsimd.partition_broadcast(rb, rrow)
            nc.vector.tensor_tensor(
                attnT[b, pair][r0:r0 + 64, :], oc, rb, AluOpType.mult)

        # warmup: projections for pair 0 as one straight PE stream
        with nc.named_scope("warmup"):
            for b in range(BPC):
                for ft in (0, 4):
                    for (c0w, cw) in CHUNKS:
                        emit_qk_chunk(b, ft, c0w, cw)
            for b in range(BPC):
                for t in range(NT):
                    emit_v_tile(b, t, 0)

        # static filler schedule: slot -> [unit, ...]
        sched = {}

        def sched_add(j, fn):
            sched.setdefault(j, []).append(fn)

        for pair in range(1, NPAIRS):
            base = 28 * (pair - 1)
            for i, (b, ft) in enumerate(
                    (b, ft) for b in range(BPC) for ft in (pair, 4 + pair)):
                for k, (c0w, cw) in enumerate(CHUNKS):
                    sched_add(base + 1 + 3 * (2 * i + k),
                              lambda b=b, ft=ft, c0w=c0w, cw=cw:
                              emit_qk_chunk(b, ft, c0w, cw))
            for b in range(BPC):
                for t in range(NT):
                    sched_add(max(0, 28 * pair + 14 * b + t - 3),
                              lambda b=b, t=t, pair=pair:
                              emit_v_tile(b, t, pair))

        oproj_q = []
        bias_fetch_q = []
        for j in range(nslots + LAG):
            for _ in range(2):
                if bias_fetch_q:
                    h, k = bias_fetch_q.pop(0)
                    fetch_bias_tile(h, k, nc.sync)
            # scores for slot j
            if j < nslots:
                pair, b, hh, kt = slots[j]
                if hh == 0 and b == 0 and kt == 0 and pair + 2 < NPAIRS:
                    bias_fetch_q.extend(
                        (h, k) for h in (2 * pair + 4, 2 * pair + 5)
                        for k in range(NT))
                emit_scores(pair, b, 2 * pair + hh, kt)
            # attn@v for slot j-LAG, then normalize at head end
            jj = j - LAG
            if jj >= 0:
                pair, b, hh, kt = slots[jj]
                h = 2 * pair + hh
                emit_attnv(pair, b, h, kt)
                if kt == NT - 1:
                    emit_normalize(pair, b, h)
                    if pair == NPAIRS - 1 and hh == 1 and b == 0:
                        oproj_q.extend(
                            lambda t=t: emit_oproj_tile(0, t,
                                                        on_act=(t % 2 == 0))
                            for t in range(NT))
            for fn in sched.pop(j, ()):
                fn()
            if oproj_q:
                oproj_q.pop(0)()

        with nc.named_scope("tail"):
            # last batch's out-projection: emit the fi<3 accumulations first
            # (their attnT inputs finished pairs ago) so the PE streams while
            # the final head's normalize chain completes.
            pend = []
            for t in range(NT):
                if len(pend) < 2:
                    pend.append((t, emit_oproj_prefix(1, t)))
            nxt = len(pend)
            while pend:
                t, ps = pend.pop(0)
                emit_oproj_finish(1, t, ps, on_act=(t % 2 == 0),
                                  split_dma=(t == NT - 1))
                if nxt < NT:
                    pend.append((nxt, emit_oproj_prefix(1, nxt)))
                    nxt += 1

    nc.compile()
    _CACHE['nc'] = nc
    return nc


def host_prep(x, w_qkv, pos_table, w_out):
    x = np.asarray(x, np.float32).reshape(B, N, C)
    wq = np.array(np.asarray(w_qkv, np.float32), copy=True)
    wq[:, :C] *= np.float32(1.0 / np.sqrt(KD))
    wq_bf = wq.astype(ml_dtypes.bfloat16)
    idx = _rel_index()
    biasT = np.ascontiguousarray(np.exp(
        np.asarray(pos_table, np.float32)[:, idx].transpose(0, 2, 1)
    )).astype(ml_dtypes.bfloat16)
    wout = np.ascontiguousarray(np.asarray(w_out, np.float32)).astype(
        ml_dtypes.bfloat16)
    in_maps = []
    for c in range(NCORES):
        xT = np.ascontiguousarray(
            x[c * BPC:(c + 1) * BPC].transpose(0, 2, 1)).astype(
                ml_dtypes.bfloat16)  # [2, 512, 784]
        in_maps.append({"xT": xT, "wqkv": wq_bf, "wout": wout, "biasT": biasT})
    return in_maps


def run(in_maps, trace=False, trace_cores=None):
    import concourse.bass_utils as bass_utils
    nc = build_nc()
    return bass_utils.run_bass_kernel_spmd(
        nc, in_maps, core_ids=list(range(NCORES)),
        trace=trace, trace_cores=trace_cores)


def kernel(x, w_qkv, pos_table, w_out):
    in_maps = host_prep(x, w_qkv, pos_table, w_out)
    res = run(in_maps)
    out = np.stack([r["out"] for r in res.results])    # [8, 2, 784, 512]
    return np.ascontiguousarray(out.reshape(B, HH, WW, C)).astype(np.float32)


# revision 9
# speedup vs baseline: 1.3901x; 1.0343x over previous
"""Trainium2 Bass kernel V3: MHSA with multi-head relative position embedding.

Data-parallel over batch: 16 batches / 8 cores = 2 per core, all 8 heads local.

V3: one flat software pipeline over 112 (head, k-tile) slots so the PE stream
never breaks (TRN2 p-states make every PE idle gap cost ~3us of half-clock).
Per slot: scores matmuls for slot j, attn@v matmuls for slot j-LAG (the trail
crosses head boundaries), plus column-budget-paced filler work (qkv projection
tiles for future pairs, out-projection once a batch finishes). The attn@v
accumulator is freed by ONE full-tile ACT copy (same ACT cost as copying just
the sumexp row, since ACT time only depends on the free dim), so op needs a
single 2-bank PSUM buffer and the normalize chain (DVE approx-recip, GPSIMD
partition-broadcast, DVE mult) runs entirely off PSUM/critical path.

PSUM: 2x scores [112,784] (2 banks each) + 1x op [65,784] (2 banks) +
2x 1-bank filler = 8 banks.
"""

import numpy as np
import ml_dtypes

B, HH, WW, C = 16, 28, 28, 512
N = HH * WW             # 784 tokens
HEADS, KD = 8, 64
NCORES, BPC = 8, 2      # 8 cores, 2 batches per core
NT, TP = 7, 112         # 784 = 7 k/token tiles of 112
CHUNKS = [(0, 512), (512, 272)]   # PSUM bank split of the 784-wide free dim
CT = 4                  # contraction tiles of 128 over C=512
NPAIRS = 4
LAG = 3                 # attn@v trails scores by this many slots

_CACHE = {}


def _rel_index():
    t = np.arange(N)
    c0, c1 = t % HH, t // HH
    return ((c0[:, None] - c0[None, :] + HH - 1)
            + (c1[:, None] - c1[None, :] + WW - 1) * (2 * HH - 1))  # [q, k]


def build_nc():
    if 'nc' in _CACHE:
        return _CACHE['nc']
    from contextlib import ExitStack
    import concourse.bacc as bacc
    import concourse.mybir as mybir
    import concourse.tile as tile
    from concourse.alu_op_type import AluOpType

    f32 = mybir.dt.float32
    bf16 = mybir.dt.bfloat16
    EXP = mybir.ActivationFunctionType.Exp

    nc = bacc.Bacc("TRN2", debug=False, enable_asserts=False)
    xT_d = nc.dram_tensor("xT", [BPC, C, N], bf16, kind="ExternalInput").ap()
    wqkv_d = nc.dram_tensor("wqkv", [C, 3 * C], bf16, kind="ExternalInput").ap()
    wout_d = nc.dram_tensor("wout", [C, C], bf16, kind="ExternalInput").ap()
    bias_d = nc.dram_tensor("biasT", [HEADS, N, N], bf16, kind="ExternalInput").ap()
    out_d = nc.dram_tensor("out", [BPC, N, C], f32, kind="ExternalOutput").ap()

    with tile.TileContext(nc) as tc, ExitStack() as ctx:
        persist = ctx.enter_context(tc.tile_pool(name="persist", bufs=1))
        bias_pool = ctx.enter_context(tc.tile_pool(name="biasp", bufs=28))
        eraw_pool = ctx.enter_context(tc.tile_pool(name="erp", bufs=3))
        esb_pool = ctx.enter_context(tc.tile_pool(name="ep", bufs=6))
        oc_pool = ctx.enter_context(tc.tile_pool(name="ocp", bufs=3))
        r_pool = ctx.enter_context(tc.tile_pool(name="rp", bufs=4))
        osb_pool = ctx.enter_context(tc.tile_pool(name="osbp", bufs=4))
        sc_psum = ctx.enter_context(tc.tile_pool(name="scp", bufs=2, space="PSUM"))
        op_psum = ctx.enter_context(tc.tile_pool(name="opp", bufs=1, space="PSUM"))
        fl_psum = ctx.enter_context(tc.tile_pool(name="flp", bufs=2, space="PSUM"))

        # ---- persistent SBUF tensors ----
        wqkv_sb = [persist.tile([128, 3 * C], bf16, tag=f"wqkv{ci}",
                                name=f"wqkv{ci}") for ci in range(CT)]
        wout_sb = [persist.tile([128, C], bf16, tag=f"wout{ci}",
                                name=f"woutw{ci}") for ci in range(CT)]
        xts = {(b, ci): persist.tile([128, N], bf16, tag=f"xT{b}_{ci}",
                                     name=f"xT{b}_{ci}")
               for b in range(BPC) for ci in range(CT)}
        qkT = {(b, ft): persist.tile([128, N], bf16, tag=f"qkT{b}_{ft}",
                                     name=f"qkT{b}_{ft}")
               for b in range(BPC) for ft in range(8)}
        vsb = {(b, t): persist.tile([TP, HEADS, KD + 1], bf16, tag=f"v{b}_{t}",
                                    name=f"v{b}_{t}")
               for b in range(BPC) for t in range(NT)}
        attnT = {(b, fi): persist.tile([128, N], bf16, tag=f"attnT{b}_{fi}",
                                       name=f"attnT{b}_{fi}")
                 for b in range(BPC) for fi in range(CT)}

        # ---- input DMAs on two issue queues (sync + gpsimd) so the DGE
        # config time (~600ns per dma_start) does not serialize the start ----
        for ci in range(CT):
            nc.sync.dma_start(xts[0, ci], xT_d[0, ci * 128:(ci + 1) * 128, :])
            nc.gpsimd.dma_start(wqkv_sb[ci], wqkv_d[ci * 128:(ci + 1) * 128, :])
        for ci in range(CT):
            nc.sync.dma_start(xts[1, ci], xT_d[1, ci * 128:(ci + 1) * 128, :])
            nc.gpsimd.dma_start(wout_sb[ci], wout_d[ci * 128:(ci + 1) * 128, :])

        bias_sb = {}

        def fetch_bias_tile(h, kt, queue):
            bt = bias_pool.tile([TP, N], bf16, tag="bias", name=f"bias{h}_{kt}")
            queue.dma_start(bt, bias_d[h, kt * TP:(kt + 1) * TP, :])
            bias_sb[h, kt] = bt

        def fetch_bias(pair, queue):
            for h in (2 * pair, 2 * pair + 1):
                for kt in range(NT):
                    fetch_bias_tile(h, kt, queue)

        fetch_bias(0, nc.sync)
        fetch_bias(1, nc.sync)

        # ---- filler work units; each returns its PE column cost ----
        def emit_qk_chunk(b, ft, c0w, cw):
            ps = fl_psum.tile([128, 512], f32, tag="fl", name=f"pj{b}_{ft}_{c0w}")
            for ci in range(CT):
                nc.tensor.matmul(
                    ps[:, 0:cw], wqkv_sb[ci][:, ft * 128:(ft + 1) * 128],
                    xts[b, ci][:, c0w:c0w + cw],
                    start=(ci == 0), stop=(ci == CT - 1))
            nc.vector.tensor_copy(qkT[b, ft][:, c0w:c0w + cw], ps[:, 0:cw])
            return CT * cw

        def emit_v_tile(b, t, pair):
            ps = fl_psum.tile([128, 512], f32, tag="fl", name=f"pv{b}_{t}_{pair}")
            f0 = 2 * C + 128 * pair
            for ci in range(CT):
                nc.tensor.matmul(
                    ps[0:TP, 0:128], xts[b, ci][:, t * TP:(t + 1) * TP],
                    wqkv_sb[ci][:, f0:f0 + 128],
                    start=(ci == 0), stop=(ci == CT - 1))
            nc.vector.tensor_copy(
                vsb[b, t][:, 2 * pair:2 * pair + 2, 0:KD],
                ps[0:TP, 0:128].rearrange("p (h d) -> p h d", h=2))
            if pair == 0:
                nc.vector.memset(vsb[b, t][:, :, KD:KD + 1], 1.0)
            return CT * 128

        def emit_oproj_prefix(b, t):
            ps = fl_psum.tile([128, 512], f32, tag="fl", name=f"po{b}_{t}")
            for fi in range(CT - 1):
                nc.tensor.matmul(
                    ps[0:TP, :], attnT[b, fi][:, t * TP:(t + 1) * TP], wout_sb[fi],
                    start=(fi == 0), stop=False)
            return ps

        def emit_oproj_finish(b, t, ps, on_act, split_dma=False):
            fi = CT - 1
            nc.tensor.matmul(
                ps[0:TP, :], attnT[b, fi][:, t * TP:(t + 1) * TP], wout_sb[fi],
                start=False, stop=True)
            osb = osb_pool.tile([TP, C], f32, tag="osb", name="osb")
            eng = nc.scalar.copy if on_act else nc.vector.tensor_copy
            if split_dma:
                eng(osb[:, 0:C // 2], ps[0:TP, 0:C // 2])
                nc.sync.dma_start(out_d[b, t * TP:(t + 1) * TP, 0:C // 2],
                                  osb[:, 0:C // 2])
                eng(osb[:, C // 2:C], ps[0:TP, C // 2:C])
                nc.sync.dma_start(out_d[b, t * TP:(t + 1) * TP, C // 2:C],
                                  osb[:, C // 2:C])
            else:
                eng(osb, ps[0:TP, :])
                nc.sync.dma_start(out_d[b, t * TP:(t + 1) * TP, :], osb)

        def emit_oproj_tile(b, t, on_act):
            ps = emit_oproj_prefix(b, t)
            emit_oproj_finish(b, t, ps, on_act)
            return CT * C

        # ---- flat slot pipeline ----
        slots = [(pair, b, hh, kt)
                 for pair in range(NPAIRS)
                 for b in range(BPC)
                 for hh in range(2)
                 for kt in range(NT)]
        nslots = len(slots)

        esbs = {}        # (head-id, kt) -> esb tile
        ops = {}         # head-id -> op psum tile

        def emit_scores(pair, b, h, kt):
            hh = h % 2
            r0 = hh * 64
            sc = sc_psum.tile([TP, N], f32, tag="sc", name=f"sc{b}_{h}_{kt}")
            for (c0w, cw) in CHUNKS:
                nc.tensor.matmul(
                    sc[:, c0w:c0w + cw],
                    qkT[b, 4 + pair][r0:r0 + 64, kt * TP:(kt + 1) * TP],
                    qkT[b, pair][r0:r0 + 64, c0w:c0w + cw],
                    start=True, stop=True)
            eraw = eraw_pool.tile([TP, N], bf16, tag="eraw",
                                  name=f"er{b}_{h}_{kt}")
            nc.scalar.activation(eraw, sc, EXP)
            esb = esb_pool.tile([TP, N], bf16, tag="e", name=f"e{b}_{h}_{kt}")
            nc.vector.tensor_tensor(esb, eraw, bias_sb[h, kt], AluOpType.mult)
            esbs[(pair, b, h), kt] = esb

        def emit_attnv(pair, b, h, kt):
            hid = (pair, b, h)
            if kt == 0:
                ops[hid] = op_psum.tile([KD + 1, N], f32, tag="op",
                                        name=f"op{b}_{h}")
            op = ops[hid]
            for (c0w, cw) in CHUNKS:
                nc.tensor.matmul(
                    op[:, c0w:c0w + cw], vsb[b, kt][:, h:h + 1, :],
                    esbs[hid, kt][:, c0w:c0w + cw],
                    start=(kt == 0), stop=(kt == NT - 1))

        def emit_normalize(pair, b, h):
            hh = h % 2
            r0 = hh * 64
            op = ops.pop((pair, b, h))
            # two ACT copies free the op psum fast; the custom-DVE recip needs
            # its input at partition 0 and the value rows copy with an even
            # partition count (the fused [65,784] variant corrupts on HW)
            srow = r_pool.tile([1, N], f32, tag="srow", name="srow")
            nc.scalar.copy(srow, op[KD:KD + 1, :])
            rrow = r_pool.tile([1, N], f32, tag="rrow", name="rrow")
            nc.vector.reciprocal_approx_fast(rrow, srow)
            oc = oc_pool.tile([KD, N], f32, tag="oc", name=f"oc{b}_{h}")
            nc.scalar.copy(oc, op[0:KD, :])
            rb = r_pool.tile([64, N], f32, tag="rb", name="rb")
            nc.gpsimd.partition_broadcast(rb, rrow)
            nc.vector.tensor_tensor(
                attnT[b, pair][r0:r0 + 64, :], oc, rb, AluOpType.mult)

        # warmup: projections for pair 0 as one straight PE stream
        with nc.named_scope("warmup"):
            for b in range(BPC):
                for ft in (0, 4):
                    for (c0w, cw) in CHUNKS:
                        emit_qk_chunk(b, ft, c0w, cw)
            for b in range(BPC):
                for t in range(NT):
                    emit_v_tile(b, t, 0)

        # static filler schedule: slot -> [unit, ...]
        sched = {}

        def sched_add(j, fn):
            sched.setdefault(j, []).append(fn)

        for pair in range(1, NPAIRS):
            base = 28 * (pair - 1)
            for i, (b, ft) in enumerate(
                    (b, ft) for b in range(BPC) for ft in (pair, 4 + pair)):
                for k, (c0w, cw) in enumerate(CHUNKS):
                    sched_add(base + 1 + 3 * (2 * i + k),
                              lambda b=b, ft=ft, c0w=c0w, cw=cw:
                              emit_qk_chunk(b, ft, c0w, cw))
            for b in range(BPC):
                for t in range(NT):
                    sched_add(max(0, 28 * pair + 14 * b + t - 3),
                              lambda b=b, t=t, pair=pair:
                              emit_v_tile(b, t, pair))

        oproj_q = []
        bias_fetch_q = []
        for j in range(nslots + LAG):
            for _ in range(2):
                if bias_fetch_q:
                    h, k = bias_fetch_q.pop(0)
                    fetch_bias_tile(h, k, nc.sync)
            # scores for slot j
            if j < nslots:
                pair, b, hh, kt = slots[j]
                if hh == 0 and b == 0 and kt == 0 and pair + 2 < NPAIRS:
                    bias_fetch_q.extend(
                        (h, k) for h in (2 * pair + 4, 2 * pair + 5)
                        for k in range(NT))
                emit_scores(pair, b, 2 * pair + hh, kt)
            # attn@v for slot j-LAG, then normalize at head end
            jj = j - LAG
            if jj >= 0:
                pair, b, hh, kt = slots[jj]
                h = 2 * pair + hh
                emit_attnv(pair, b, h, kt)
                if kt == NT - 1:
                    emit_normalize(pair, b, h)
                    if pair == NPAIRS - 1 and hh == 1 and b == 0:
                        oproj_q.extend(
                            lambda t=t: emit_oproj_tile(0, t, on_act=False)
                            for t in range(NT))
            for fn in sched.pop(j, ()):
                fn()
            if oproj_q:
                oproj_q.pop(0)()

        with nc.named_scope("tail"):
            # last batch's out-projection: emit the fi<3 accumulations first
            # (their attnT inputs finished pairs ago) so the PE streams while
            # the final head's normalize chain completes.
            pend = []
            for t in range(NT):
                if len(pend) < 2:
                    pend.append((t, emit_oproj_prefix(1, t)))
            nxt = len(pend)
            while pend:
                t, ps = pend.pop(0)
                emit_oproj_finish(1, t, ps, on_act=True,
                                  split_dma=(t == NT - 1))
                if nxt < NT:
                    pend.append((nxt, emit_oproj_prefix(1, nxt)))
                    nxt += 1

    nc.compile()
    _CACHE['nc'] = nc
    return nc


def host_prep(x, w_qkv, pos_table, w_out):
    x = np.asarray(x, np.float32).reshape(B, N, C)
    wq = np.array(np.asarray(w_qkv, np.float32), copy=True)
    wq[:, :C] *= np.float32(1.0 / np.sqrt(KD))
    wq_bf = wq.astype(ml_dtypes.bfloat16)
    idx = _rel_index()
    biasT = np.ascontiguousarray(np.exp(
        np.asarray(pos_table, np.float32)[:, idx].transpose(0, 2, 1)
    )).astype(ml_dtypes.bfloat16)
    wout = np.ascontiguousarray(np.asarray(w_out, np.float32)).astype(
        ml_dtypes.bfloat16)
    in_maps = []
    for c in range(NCORES):
        xT = np.ascontiguousarray(
            x[c * BPC:(c + 1) * BPC].transpose(0, 2, 1)).astype(
                ml_dtypes.bfloat16)  # [2, 512, 784]
        in_maps.append({"xT": xT, "wqkv": wq_bf, "wout": wout, "biasT": biasT})
    return in_maps


def run(in_maps, trace=False, trace_cores=None):
    import concourse.bass_utils as bass_utils
    nc = build_nc()
    return bass_utils.run_bass_kernel_spmd(
        nc, in_maps, core_ids=list(range(NCORES)),
        trace=trace, trace_cores=trace_cores)


def kernel(x, w_qkv, pos_table, w_out):
    in_maps = host_prep(x, w_qkv, pos_table, w_out)
    res = run(in_maps)
    out = np.stack([r["out"] for r in res.results])    # [8, 2, 784, 512]
    return np.ascontiguousarray(out.reshape(B, HH, WW, C)).astype(np.float32)


# revision 10
# speedup vs baseline: 1.3985x; 1.0061x over previous
"""Trainium2 Bass kernel V3: MHSA with multi-head relative position embedding.

Data-parallel over batch: 16 batches / 8 cores = 2 per core, all 8 heads local.

V3: one flat software pipeline over 112 (head, k-tile) slots so the PE stream
never breaks (TRN2 p-states make every PE idle gap cost ~3us of half-clock).
Per slot: scores matmuls for slot j, attn@v matmuls for slot j-LAG (the trail
crosses head boundaries), plus column-budget-paced filler work (qkv projection
tiles for future pairs, out-projection once a batch finishes). The attn@v
accumulator is freed by ONE full-tile ACT copy (same ACT cost as copying just
the sumexp row, since ACT time only depends on the free dim), so op needs a
single 2-bank PSUM buffer and the normalize chain (DVE approx-recip, GPSIMD
partition-broadcast, DVE mult) runs entirely off PSUM/critical path.

PSUM: 2x scores [112,784] (2 banks each) + 1x op [65,784] (2 banks) +
2x 1-bank filler = 8 banks.
"""

import numpy as np
import ml_dtypes

B, HH, WW, C = 16, 28, 28, 512
N = HH * WW             # 784 tokens
HEADS, KD = 8, 64
NCORES, BPC = 8, 2      # 8 cores, 2 batches per core
NT, TP = 7, 112         # 784 = 7 k/token tiles of 112
CHUNKS = [(0, 512), (512, 272)]   # PSUM bank split of the 784-wide free dim
CT = 4                  # contraction tiles of 128 over C=512
NPAIRS = 4
LAG = 3                 # attn@v trails scores by this many slots

_CACHE = {}


def _rel_index():
    t = np.arange(N)
    c0, c1 = t % HH, t // HH
    return ((c0[:, None] - c0[None, :] + HH - 1)
            + (c1[:, None] - c1[None, :] + WW - 1) * (2 * HH - 1))  # [q, k]


def build_nc():
    if 'nc' in _CACHE:
        return _CACHE['nc']
    from contextlib import ExitStack
    import concourse.bacc as bacc
    import concourse.mybir as mybir
    import concourse.tile as tile
    from concourse.alu_op_type import AluOpType

    f32 = mybir.dt.float32
    bf16 = mybir.dt.bfloat16
    EXP = mybir.ActivationFunctionType.Exp

    nc = bacc.Bacc("TRN2", debug=False, enable_asserts=False)
    xT_d = nc.dram_tensor("xT", [BPC, C, N], bf16, kind="ExternalInput").ap()
    wqkv_d = nc.dram_tensor("wqkv", [C, 3 * C], bf16, kind="ExternalInput").ap()
    wout_d = nc.dram_tensor("wout", [C, C], bf16, kind="ExternalInput").ap()
    bias_d = nc.dram_tensor("biasT", [HEADS, N, N], bf16, kind="ExternalInput").ap()
    out_d = nc.dram_tensor("out", [BPC, N, C], f32, kind="ExternalOutput").ap()

    with tile.TileContext(nc) as tc, ExitStack() as ctx:
        persist = ctx.enter_context(tc.tile_pool(name="persist", bufs=1))
        bias_pool = ctx.enter_context(tc.tile_pool(name="biasp", bufs=28))
        eraw_pool = ctx.enter_context(tc.tile_pool(name="erp", bufs=4))
        esb_pool = ctx.enter_context(tc.tile_pool(name="ep", bufs=8))
        oc_pool = ctx.enter_context(tc.tile_pool(name="ocp", bufs=3))
        r_pool = ctx.enter_context(tc.tile_pool(name="rp", bufs=4))
        osb_pool = ctx.enter_context(tc.tile_pool(name="osbp", bufs=4))
        sc_psum = ctx.enter_context(tc.tile_pool(name="scp", bufs=2, space="PSUM"))
        op_psum = ctx.enter_context(tc.tile_pool(name="opp", bufs=1, space="PSUM"))
        fl_psum = ctx.enter_context(tc.tile_pool(name="flp", bufs=2, space="PSUM"))

        # ---- persistent SBUF tensors ----
        wqkv_sb = [persist.tile([128, 3 * C], bf16, tag=f"wqkv{ci}",
                                name=f"wqkv{ci}") for ci in range(CT)]
        wout_sb = [persist.tile([128, C], bf16, tag=f"wout{ci}",
                                name=f"woutw{ci}") for ci in range(CT)]
        xts = {(b, ci): persist.tile([128, N], bf16, tag=f"xT{b}_{ci}",
                                     name=f"xT{b}_{ci}")
               for b in range(BPC) for ci in range(CT)}
        qkT = {(b, ft): persist.tile([128, N], bf16, tag=f"qkT{b}_{ft}",
                                     name=f"qkT{b}_{ft}")
               for b in range(BPC) for ft in range(8)}
        vsb = {(b, t): persist.tile([TP, HEADS, KD + 1], bf16, tag=f"v{b}_{t}",
                                    name=f"v{b}_{t}")
               for b in range(BPC) for t in range(NT)}
        attnT = {(b, fi): persist.tile([128, N], bf16, tag=f"attnT{b}_{fi}",
                                       name=f"attnT{b}_{fi}")
                 for b in range(BPC) for fi in range(CT)}

        # ---- input DMAs on two issue queues (sync + gpsimd) so the DGE
        # config time (~600ns per dma_start) does not serialize the start ----
        for ci in range(CT):
            nc.sync.dma_start(xts[0, ci], xT_d[0, ci * 128:(ci + 1) * 128, :])
            nc.gpsimd.dma_start(wqkv_sb[ci], wqkv_d[ci * 128:(ci + 1) * 128, :])
        for ci in range(CT):
            nc.sync.dma_start(xts[1, ci], xT_d[1, ci * 128:(ci + 1) * 128, :])
            nc.gpsimd.dma_start(wout_sb[ci], wout_d[ci * 128:(ci + 1) * 128, :])

        bias_sb = {}

        def fetch_bias_tile(h, kt, queue):
            bt = bias_pool.tile([TP, N], bf16, tag="bias", name=f"bias{h}_{kt}")
            queue.dma_start(bt, bias_d[h, kt * TP:(kt + 1) * TP, :])
            bias_sb[h, kt] = bt

        def fetch_bias(pair, queue):
            for h in (2 * pair, 2 * pair + 1):
                for kt in range(NT):
                    fetch_bias_tile(h, kt, queue)

        fetch_bias(0, nc.sync)
        fetch_bias(1, nc.sync)

        # ---- filler work units; each returns its PE column cost ----
        def emit_qk_chunk(b, ft, c0w, cw, on_act=False):
            ps = fl_psum.tile([128, 512], f32, tag="fl", name=f"pj{b}_{ft}_{c0w}")
            for ci in range(CT):
                nc.tensor.matmul(
                    ps[:, 0:cw], wqkv_sb[ci][:, ft * 128:(ft + 1) * 128],
                    xts[b, ci][:, c0w:c0w + cw],
                    start=(ci == 0), stop=(ci == CT - 1))
            eng = nc.scalar.copy if on_act else nc.vector.tensor_copy
            eng(qkT[b, ft][:, c0w:c0w + cw], ps[:, 0:cw])
            return CT * cw

        def emit_v_tile(b, t, pair, on_act=False):
            ps = fl_psum.tile([128, 512], f32, tag="fl", name=f"pv{b}_{t}_{pair}")
            f0 = 2 * C + 128 * pair
            for ci in range(CT):
                nc.tensor.matmul(
                    ps[0:TP, 0:128], xts[b, ci][:, t * TP:(t + 1) * TP],
                    wqkv_sb[ci][:, f0:f0 + 128],
                    start=(ci == 0), stop=(ci == CT - 1))
            eng = nc.scalar.copy if on_act else nc.vector.tensor_copy
            eng(vsb[b, t][:, 2 * pair:2 * pair + 2, 0:KD],
                ps[0:TP, 0:128].rearrange("p (h d) -> p h d", h=2))
            if pair == 0:
                nc.vector.memset(vsb[b, t][:, :, KD:KD + 1], 1.0)
            return CT * 128

        def emit_oproj_prefix(b, t):
            ps = fl_psum.tile([128, 512], f32, tag="fl", name=f"po{b}_{t}")
            for fi in range(CT - 1):
                nc.tensor.matmul(
                    ps[0:TP, :], attnT[b, fi][:, t * TP:(t + 1) * TP], wout_sb[fi],
                    start=(fi == 0), stop=False)
            return ps

        def emit_oproj_finish(b, t, ps, on_act, split_dma=False):
            fi = CT - 1
            nc.tensor.matmul(
                ps[0:TP, :], attnT[b, fi][:, t * TP:(t + 1) * TP], wout_sb[fi],
                start=False, stop=True)
            osb = osb_pool.tile([TP, C], f32, tag="osb", name="osb")
            eng = nc.scalar.copy if on_act else nc.vector.tensor_copy
            if split_dma:
                eng(osb[:, 0:C // 2], ps[0:TP, 0:C // 2])
                nc.sync.dma_start(out_d[b, t * TP:(t + 1) * TP, 0:C // 2],
                                  osb[:, 0:C // 2])
                eng(osb[:, C // 2:C], ps[0:TP, C // 2:C])
                nc.sync.dma_start(out_d[b, t * TP:(t + 1) * TP, C // 2:C],
                                  osb[:, C // 2:C])
            else:
                eng(osb, ps[0:TP, :])
                nc.sync.dma_start(out_d[b, t * TP:(t + 1) * TP, :], osb)

        def emit_oproj_tile(b, t, on_act):
            ps = emit_oproj_prefix(b, t)
            emit_oproj_finish(b, t, ps, on_act)
            return CT * C

        # ---- flat slot pipeline ----
        slots = [(pair, b, hh, kt)
                 for pair in range(NPAIRS)
                 for b in range(BPC)
                 for hh in range(2)
                 for kt in range(NT)]
        nslots = len(slots)

        esbs = {}        # (head-id, kt) -> esb tile
        ops = {}         # head-id -> op psum tile

        def emit_scores(pair, b, h, kt):
            hh = h % 2
            r0 = hh * 64
            sc = sc_psum.tile([TP, N], f32, tag="sc", name=f"sc{b}_{h}_{kt}")
            for (c0w, cw) in CHUNKS:
                nc.tensor.matmul(
                    sc[:, c0w:c0w + cw],
                    qkT[b, 4 + pair][r0:r0 + 64, kt * TP:(kt + 1) * TP],
                    qkT[b, pair][r0:r0 + 64, c0w:c0w + cw],
                    start=True, stop=True)
            eraw = eraw_pool.tile([TP, N], bf16, tag="eraw",
                                  name=f"er{b}_{h}_{kt}")
            nc.scalar.activation(eraw, sc, EXP)
            esb = esb_pool.tile([TP, N], bf16, tag="e", name=f"e{b}_{h}_{kt}")
            nc.vector.tensor_tensor(esb, eraw, bias_sb[h, kt], AluOpType.mult)
            esbs[(pair, b, h), kt] = esb

        def emit_attnv(pair, b, h, kt):
            hid = (pair, b, h)
            if kt == 0:
                ops[hid] = op_psum.tile([KD + 1, N], f32, tag="op",
                                        name=f"op{b}_{h}")
            op = ops[hid]
            for (c0w, cw) in CHUNKS:
                nc.tensor.matmul(
                    op[:, c0w:c0w + cw], vsb[b, kt][:, h:h + 1, :],
                    esbs[hid, kt][:, c0w:c0w + cw],
                    start=(kt == 0), stop=(kt == NT - 1))

        def emit_normalize(pair, b, h):
            hh = h % 2
            r0 = hh * 64
            op = ops.pop((pair, b, h))
            # two ACT copies free the op psum fast; the custom-DVE recip needs
            # its input at partition 0 and the value rows copy with an even
            # partition count (the fused [65,784] variant corrupts on HW)
            srow = r_pool.tile([1, N], f32, tag="srow", name="srow")
            nc.scalar.copy(srow, op[KD:KD + 1, :])
            rrow = r_pool.tile([1, N], f32, tag="rrow", name="rrow")
            nc.vector.reciprocal_approx_fast(rrow, srow)
            oc = oc_pool.tile([KD, N], f32, tag="oc", name=f"oc{b}_{h}")
            if (2 * pair + b) % 3 == 2:   # balance: some op copies on DVE
                nc.vector.tensor_copy(oc, op[0:KD, :])
            else:
                nc.scalar.copy(oc, op[0:KD, :])
            rb = r_pool.tile([64, N], f32, tag="rb", name="rb")
            nc.gpsimd.partition_broadcast(rb, rrow)
            nc.vector.tensor_tensor(
                attnT[b, pair][r0:r0 + 64, :], oc, rb, AluOpType.mult)

        # warmup: projections for pair 0 as one straight PE stream
        with nc.named_scope("warmup"):
            i = 0
            for b in range(BPC):
                for ft in (0, 4):
                    for (c0w, cw) in CHUNKS:
                        emit_qk_chunk(b, ft, c0w, cw, on_act=(i % 2 == 0))
                        i += 1
            for b in range(BPC):
                for t in range(NT):
                    emit_v_tile(b, t, 0)

        # static filler schedule: slot -> [unit, ...]
        sched = {}

        def sched_add(j, fn):
            sched.setdefault(j, []).append(fn)

        for pair in range(1, NPAIRS):
            base = 28 * (pair - 1)
            for i, (b, ft) in enumerate(
                    (b, ft) for b in range(BPC) for ft in (pair, 4 + pair)):
                for k, (c0w, cw) in enumerate(CHUNKS):
                    sched_add(base + 1 + 3 * (2 * i + k),
                              lambda b=b, ft=ft, c0w=c0w, cw=cw:
                              emit_qk_chunk(b, ft, c0w, cw))
            for b in range(BPC):
                for t in range(NT):
                    sched_add(max(0, 28 * pair + 14 * b + t - 3),
                              lambda b=b, t=t, pair=pair:
                              emit_v_tile(b, t, pair))

        oproj_q = []
        bias_fetch_q = []
        for j in range(nslots + LAG):
            for _ in range(2):
                if bias_fetch_q:
                    h, k = bias_fetch_q.pop(0)
                    fetch_bias_tile(h, k, nc.sync)
            # scores for slot j
            if j < nslots:
                pair, b, hh, kt = slots[j]
                if hh == 0 and b == 0 and kt == 0 and pair + 2 < NPAIRS:
                    bias_fetch_q.extend(
                        (h, k) for h in (2 * pair + 4, 2 * pair + 5)
                        for k in range(NT))
                emit_scores(pair, b, 2 * pair + hh, kt)
            # attn@v for slot j-LAG, then normalize at head end
            jj = j - LAG
            if jj >= 0:
                pair, b, hh, kt = slots[jj]
                h = 2 * pair + hh
                emit_attnv(pair, b, h, kt)
                if kt == NT - 1:
                    emit_normalize(pair, b, h)
                    if pair == NPAIRS - 1 and hh == 1 and b == 0:
                        oproj_q.extend(
                            lambda t=t: emit_oproj_tile(0, t, on_act=False)
                            for t in range(NT))
            for fn in sched.pop(j, ()):
                fn()
            if oproj_q:
                oproj_q.pop(0)()

        with nc.named_scope("tail"):
            # last batch's out-projection: emit the fi<3 accumulations first
            # (their attnT inputs finished pairs ago) so the PE streams while
            # the final head's normalize chain completes.
            pend = []
            for t in range(NT):
                if len(pend) < 2:
                    pend.append((t, emit_oproj_prefix(1, t)))
            nxt = len(pend)
            while pend:
                t, ps = pend.pop(0)
                emit_oproj_finish(1, t, ps, on_act=True,
                                  split_dma=(t == NT - 1))
                if nxt < NT:
                    pend.append((nxt, emit_oproj_prefix(1, nxt)))
                    nxt += 1

    nc.compile()
    _CACHE['nc'] = nc
    return nc


def host_prep(x, w_qkv, pos_table, w_out):
    x = np.asarray(x, np.float32).reshape(B, N, C)
    wq = np.array(np.asarray(w_qkv, np.float32), copy=True)
    wq[:, :C] *= np.float32(1.0 / np.sqrt(KD))
    wq_bf = wq.astype(ml_dtypes.bfloat16)
    idx = _rel_index()
    biasT = np.ascontiguousarray(np.exp(
        np.asarray(pos_table, np.float32)[:, idx].transpose(0, 2, 1)
    )).astype(ml_dtypes.bfloat16)
    wout = np.ascontiguousarray(np.asarray(w_out, np.float32)).astype(
        ml_dtypes.bfloat16)
    in_maps = []
    for c in range(NCORES):
        xT = np.ascontiguousarray(
            x[c * BPC:(c + 1) * BPC].transpose(0, 2, 1)).astype(
                ml_dtypes.bfloat16)  # [2, 512, 784]
        in_maps.append({"xT": xT, "wqkv": wq_bf, "wout": wout, "biasT": biasT})
    return in_maps


def run(in_maps, trace=False, trace_cores=None):
    import concourse.bass_utils as bass_utils
    nc = build_nc()
    return bass_utils.run_bass_kernel_spmd(
        nc, in_maps, core_ids=list(range(NCORES)),
        trace=trace, trace_cores=trace_cores)


def kernel(x, w_qkv, pos_table, w_out):
    in_maps = host_prep(x, w_qkv, pos_table, w_out)
    res = run(in_maps)
    out = np.stack([r["out"] for r in res.results])    # [8, 2, 784, 512]
    return np.ascontiguousarray(out.reshape(B, HH, WW, C)).astype(np.float32)


# revision 12
# speedup vs baseline: 1.4058x; 1.0052x over previous
"""Trainium2 Bass kernel V3: MHSA with multi-head relative position embedding.

Data-parallel over batch: 16 batches / 8 cores = 2 per core, all 8 heads local.

V3: one flat software pipeline over 112 (head, k-tile) slots so the PE stream
never breaks (TRN2 p-states make every PE idle gap cost ~3us of half-clock).
Per slot: scores matmuls for slot j, attn@v matmuls for slot j-LAG (the trail
crosses head boundaries), plus column-budget-paced filler work (qkv projection
tiles for future pairs, out-projection once a batch finishes). The attn@v
accumulator is freed by ONE full-tile ACT copy (same ACT cost as copying just
the sumexp row, since ACT time only depends on the free dim), so op needs a
single 2-bank PSUM buffer and the normalize chain (DVE approx-recip, GPSIMD
partition-broadcast, DVE mult) runs entirely off PSUM/critical path.

PSUM: 2x scores [112,784] (2 banks each) + 1x op [65,784] (2 banks) +
2x 1-bank filler = 8 banks.
"""

import numpy as np
import ml_dtypes

B, HH, WW, C = 16, 28, 28, 512
N = HH * WW             # 784 tokens
HEADS, KD = 8, 64
NCORES, BPC = 8, 2      # 8 cores, 2 batches per core
NT, TP = 7, 112         # 784 = 7 k/token tiles of 112
CHUNKS = [(0, 512), (512, 272)]   # PSUM bank split of the 784-wide free dim
CT = 4                  # contraction tiles of 128 over C=512
NPAIRS = 4
LAG = 3                 # attn@v trails scores by this many slots

_CACHE = {}


def _rel_index():
    t = np.arange(N)
    c0, c1 = t % HH, t // HH
    return ((c0[:, None] - c0[None, :] + HH - 1)
            + (c1[:, None] - c1[None, :] + WW - 1) * (2 * HH - 1))  # [q, k]


def build_nc():
    if 'nc' in _CACHE:
        return _CACHE['nc']
    from contextlib import ExitStack
    import concourse.bacc as bacc
    import concourse.mybir as mybir
    import concourse.tile as tile
    from concourse.alu_op_type import AluOpType

    f32 = mybir.dt.float32
    bf16 = mybir.dt.bfloat16
    EXP = mybir.ActivationFunctionType.Exp

    nc = bacc.Bacc("TRN2", debug=False, enable_asserts=False)
    xT_d = nc.dram_tensor("xT", [BPC, C, N], bf16, kind="ExternalInput").ap()
    wqkv_d = nc.dram_tensor("wqkv", [C, 3 * C], bf16, kind="ExternalInput").ap()
    wout_d = nc.dram_tensor("wout", [C, C], bf16, kind="ExternalInput").ap()
    bias_d = nc.dram_tensor("biasT", [HEADS, N, N], bf16, kind="ExternalInput").ap()
    out_d = nc.dram_tensor("out", [BPC, N, C], f32, kind="ExternalOutput").ap()

    with tile.TileContext(nc) as tc, ExitStack() as ctx:
        persist = ctx.enter_context(tc.tile_pool(name="persist", bufs=1))
        bias_pool = ctx.enter_context(tc.tile_pool(name="biasp", bufs=28))
        eraw_pool = ctx.enter_context(tc.tile_pool(name="erp", bufs=4))
        esb_pool = ctx.enter_context(tc.tile_pool(name="ep", bufs=8))
        oc_pool = ctx.enter_context(tc.tile_pool(name="ocp", bufs=3))
        r_pool = ctx.enter_context(tc.tile_pool(name="rp", bufs=4))
        osb_pool = ctx.enter_context(tc.tile_pool(name="osbp", bufs=4))
        sc_psum = ctx.enter_context(tc.tile_pool(name="scp", bufs=2, space="PSUM"))
        op_psum = ctx.enter_context(tc.tile_pool(name="opp", bufs=1, space="PSUM"))
        fl_psum = ctx.enter_context(tc.tile_pool(name="flp", bufs=2, space="PSUM"))

        # ---- persistent SBUF tensors ----
        wqkv_sb = [persist.tile([128, 3 * C], bf16, tag=f"wqkv{ci}",
                                name=f"wqkv{ci}") for ci in range(CT)]
        wout_sb = [persist.tile([128, C], bf16, tag=f"wout{ci}",
                                name=f"woutw{ci}") for ci in range(CT)]
        xts = {(b, ci): persist.tile([128, N], bf16, tag=f"xT{b}_{ci}",
                                     name=f"xT{b}_{ci}")
               for b in range(BPC) for ci in range(CT)}
        qkT = {(b, ft): persist.tile([128, N], bf16, tag=f"qkT{b}_{ft}",
                                     name=f"qkT{b}_{ft}")
               for b in range(BPC) for ft in range(8)}
        vsb = {(b, t): persist.tile([TP, HEADS, KD + 1], bf16, tag=f"v{b}_{t}",
                                    name=f"v{b}_{t}")
               for b in range(BPC) for t in range(NT)}
        attnT = {(b, fi): persist.tile([128, N], bf16, tag=f"attnT{b}_{fi}",
                                       name=f"attnT{b}_{fi}")
                 for b in range(BPC) for fi in range(CT)}

        # ---- input DMAs on two issue queues (sync + gpsimd) so the DGE
        # config time (~600ns per dma_start) does not serialize the start ----
        for ci in range(CT):
            nc.sync.dma_start(xts[0, ci], xT_d[0, ci * 128:(ci + 1) * 128, :])
            nc.gpsimd.dma_start(wqkv_sb[ci], wqkv_d[ci * 128:(ci + 1) * 128, :])
        for ci in range(CT):
            nc.sync.dma_start(xts[1, ci], xT_d[1, ci * 128:(ci + 1) * 128, :])
            nc.gpsimd.dma_start(wout_sb[ci], wout_d[ci * 128:(ci + 1) * 128, :])

        bias_sb = {}

        def fetch_bias_tile(h, kt, queue):
            bt = bias_pool.tile([TP, N], bf16, tag="bias", name=f"bias{h}_{kt}")
            queue.dma_start(bt, bias_d[h, kt * TP:(kt + 1) * TP, :])
            bias_sb[h, kt] = bt

        def fetch_bias(pair, queue):
            for h in (2 * pair, 2 * pair + 1):
                for kt in range(NT):
                    fetch_bias_tile(h, kt, queue)

        fetch_bias(0, nc.sync)
        fetch_bias(1, nc.sync)

        # ---- filler work units; each returns its PE column cost ----
        def emit_qk_chunk(b, ft, c0w, cw, on_act=False):
            ps = fl_psum.tile([128, 512], f32, tag="fl", name=f"pj{b}_{ft}_{c0w}")
            for ci in range(CT):
                nc.tensor.matmul(
                    ps[:, 0:cw], wqkv_sb[ci][:, ft * 128:(ft + 1) * 128],
                    xts[b, ci][:, c0w:c0w + cw],
                    start=(ci == 0), stop=(ci == CT - 1))
            eng = nc.scalar.copy if on_act else nc.vector.tensor_copy
            eng(qkT[b, ft][:, c0w:c0w + cw], ps[:, 0:cw])
            return CT * cw

        def emit_v_tile(b, t, pair, on_act=False):
            ps = fl_psum.tile([128, 512], f32, tag="fl", name=f"pv{b}_{t}_{pair}")
            f0 = 2 * C + 128 * pair
            for ci in range(CT):
                nc.tensor.matmul(
                    ps[0:TP, 0:128], xts[b, ci][:, t * TP:(t + 1) * TP],
                    wqkv_sb[ci][:, f0:f0 + 128],
                    start=(ci == 0), stop=(ci == CT - 1))
            eng = nc.scalar.copy if on_act else nc.vector.tensor_copy
            eng(vsb[b, t][:, 2 * pair:2 * pair + 2, 0:KD],
                ps[0:TP, 0:128].rearrange("p (h d) -> p h d", h=2))
            if pair == 0:
                nc.vector.memset(vsb[b, t][:, :, KD:KD + 1], 1.0)
            return CT * 128

        def emit_oproj_prefix(b, t):
            ps = fl_psum.tile([128, 512], f32, tag="fl", name=f"po{b}_{t}")
            for fi in range(CT - 1):
                nc.tensor.matmul(
                    ps[0:TP, :], attnT[b, fi][:, t * TP:(t + 1) * TP], wout_sb[fi],
                    start=(fi == 0), stop=False)
            return ps

        def emit_oproj_finish(b, t, ps, on_act, split_dma=False):
            fi = CT - 1
            nc.tensor.matmul(
                ps[0:TP, :], attnT[b, fi][:, t * TP:(t + 1) * TP], wout_sb[fi],
                start=False, stop=True)
            osb = osb_pool.tile([TP, C], f32, tag="osb", name="osb")
            eng = nc.scalar.copy if on_act else nc.vector.tensor_copy
            if split_dma:
                eng(osb[:, 0:C // 2], ps[0:TP, 0:C // 2])
                nc.sync.dma_start(out_d[b, t * TP:(t + 1) * TP, 0:C // 2],
                                  osb[:, 0:C // 2])
                eng(osb[:, C // 2:C], ps[0:TP, C // 2:C])
                nc.sync.dma_start(out_d[b, t * TP:(t + 1) * TP, C // 2:C],
                                  osb[:, C // 2:C])
            else:
                eng(osb, ps[0:TP, :])
                nc.sync.dma_start(out_d[b, t * TP:(t + 1) * TP, :], osb)

        def emit_oproj_tile(b, t, on_act):
            ps = emit_oproj_prefix(b, t)
            emit_oproj_finish(b, t, ps, on_act)
            return CT * C

        # ---- flat slot pipeline ----
        slots = [(pair, b, hh, kt)
                 for pair in range(NPAIRS)
                 for b in range(BPC)
                 for hh in range(2)
                 for kt in range(NT)]
        nslots = len(slots)

        esbs = {}        # (head-id, kt) -> esb tile
        ops = {}         # head-id -> op psum tile

        def emit_scores(pair, b, h, kt):
            hh = h % 2
            r0 = hh * 64
            sc = sc_psum.tile([TP, N], f32, tag="sc", name=f"sc{b}_{h}_{kt}")
            for (c0w, cw) in CHUNKS:
                nc.tensor.matmul(
                    sc[:, c0w:c0w + cw],
                    qkT[b, 4 + pair][r0:r0 + 64, kt * TP:(kt + 1) * TP],
                    qkT[b, pair][r0:r0 + 64, c0w:c0w + cw],
                    start=True, stop=True)
            eraw = eraw_pool.tile([TP, N], bf16, tag="eraw",
                                  name=f"er{b}_{h}_{kt}")
            nc.scalar.activation(eraw, sc, EXP)
            esb = esb_pool.tile([TP, N], bf16, tag="e", name=f"e{b}_{h}_{kt}")
            nc.vector.tensor_tensor(esb, eraw, bias_sb[h, kt], AluOpType.mult)
            esbs[(pair, b, h), kt] = esb

        def emit_attnv(pair, b, h, kt):
            hid = (pair, b, h)
            if kt == 0:
                ops[hid] = op_psum.tile([KD + 1, N], f32, tag="op",
                                        name=f"op{b}_{h}")
            op = ops[hid]
            for (c0w, cw) in CHUNKS:
                nc.tensor.matmul(
                    op[:, c0w:c0w + cw], vsb[b, kt][:, h:h + 1, :],
                    esbs[hid, kt][:, c0w:c0w + cw],
                    start=(kt == 0), stop=(kt == NT - 1))

        def emit_normalize(pair, b, h):
            hh = h % 2
            r0 = hh * 64
            op = ops.pop((pair, b, h))
            # two ACT copies free the op psum fast; the custom-DVE recip needs
            # its input at partition 0 and the value rows copy with an even
            # partition count (the fused [65,784] variant corrupts on HW)
            srow = r_pool.tile([1, N], f32, tag="srow", name="srow")
            nc.scalar.copy(srow, op[KD:KD + 1, :])
            rrow = r_pool.tile([1, N], f32, tag="rrow", name="rrow")
            nc.vector.reciprocal_approx_fast(rrow, srow)
            oc = oc_pool.tile([KD, N], f32, tag="oc", name=f"oc{b}_{h}")
            if (2 * pair + b) % 3 == 2:   # balance: some op copies on DVE
                nc.vector.tensor_copy(oc, op[0:KD, :])
            else:
                nc.scalar.copy(oc, op[0:KD, :])
            rb = r_pool.tile([64, N], f32, tag="rb", name="rb")
            nc.gpsimd.partition_broadcast(rb, rrow)
            nc.vector.tensor_tensor(
                attnT[b, pair][r0:r0 + 64, :], oc, rb, AluOpType.mult)

        # warmup: projections for pair 0 as one straight PE stream
        with nc.named_scope("warmup"):
            i = 0
            for b in range(BPC):
                for ft in (0, 4):
                    for (c0w, cw) in CHUNKS:
                        emit_qk_chunk(b, ft, c0w, cw, on_act=(i % 2 == 0))
                        i += 1
            for b in range(BPC):
                for t in range(NT):
                    emit_v_tile(b, t, 0)

        # static filler schedule: slot -> [unit, ...]
        sched = {}

        def sched_add(j, fn):
            sched.setdefault(j, []).append(fn)

        for pair in range(1, NPAIRS):
            base = 28 * (pair - 1)
            for i, (b, ft) in enumerate(
                    (b, ft) for b in range(BPC) for ft in (pair, 4 + pair)):
                for k, (c0w, cw) in enumerate(CHUNKS):
                    sched_add(base + 1 + 3 * (2 * i + k),
                              lambda b=b, ft=ft, c0w=c0w, cw=cw:
                              emit_qk_chunk(b, ft, c0w, cw))
            for b in range(BPC):
                for t in range(NT):
                    sched_add(max(0, 28 * pair + 14 * b + t - 3),
                              lambda b=b, t=t, pair=pair:
                              emit_v_tile(b, t, pair))

        oproj_q = []
        bias_fetch_q = []
        for j in range(nslots + LAG):
            for _ in range(2):
                if bias_fetch_q:
                    h, k = bias_fetch_q.pop(0)
                    fetch_bias_tile(h, k, nc.sync)
            # scores for slot j
            if j < nslots:
                pair, b, hh, kt = slots[j]
                if hh == 0 and b == 0 and kt == 0 and pair + 2 < NPAIRS:
                    bias_fetch_q.extend(
                        (h, k) for h in (2 * pair + 4, 2 * pair + 5)
                        for k in range(NT))
                emit_scores(pair, b, 2 * pair + hh, kt)
            # attn@v for slot j-LAG, then normalize at head end
            jj = j - LAG
            if jj >= 0:
                pair, b, hh, kt = slots[jj]
                h = 2 * pair + hh
                emit_attnv(pair, b, h, kt)
                if kt == NT - 1:
                    emit_normalize(pair, b, h)
                    if pair == NPAIRS - 1 and hh == 1 and b == 0:
                        oproj_q.extend(
                            lambda t=t: emit_oproj_tile(0, t, on_act=False)
                            for t in range(NT))
            for fn in sched.pop(j, ()):
                fn()
            if oproj_q:
                oproj_q.pop(0)()

        with nc.named_scope("tail"):
            # last batch's out-projection: emit the fi<3 accumulations first
            # (their attnT inputs finished pairs ago) so the PE streams while
            # the final head's normalize chain completes.
            pend = []
            for t in range(NT):
                if len(pend) < 2:
                    pend.append((t, emit_oproj_prefix(1, t)))
            nxt = len(pend)
            while pend:
                t, ps = pend.pop(0)
                emit_oproj_finish(1, t, ps, on_act=True,
                                  split_dma=(t == NT - 1))
                if nxt < NT:
                    pend.append((nxt, emit_oproj_prefix(1, nxt)))
                    nxt += 1

    nc.compile()
    _CACHE['nc'] = nc
    return nc


def host_prep(x, w_qkv, pos_table, w_out):
    x = np.asarray(x, np.float32).reshape(B, N, C)
    wq = np.array(np.asarray(w_qkv, np.float32), copy=True)
    wq[:, :C] *= np.float32(1.0 / np.sqrt(KD))
    wq_bf = wq.astype(ml_dtypes.bfloat16)
    idx = _rel_index()
    biasT = np.ascontiguousarray(np.exp(
        np.asarray(pos_table, np.float32)[:, idx].transpose(0, 2, 1)
    )).astype(ml_dtypes.bfloat16)
    wout = np.ascontiguousarray(np.asarray(w_out, np.float32)).astype(
        ml_dtypes.bfloat16)
    in_maps = []
    for c in range(NCORES):
        xT = np.ascontiguousarray(
            x[c * BPC:(c + 1) * BPC].transpose(0, 2, 1)).astype(
                ml_dtypes.bfloat16)  # [2, 512, 784]
        in_maps.append({"xT": xT, "wqkv": wq_bf, "wout": wout, "biasT": biasT})
    return in_maps


def run(in_maps, trace=False, trace_cores=None):
    import concourse.bass_utils as bass_utils
    nc = build_nc()
    return bass_utils.run_bass_kernel_spmd(
        nc, in_maps, core_ids=list(range(NCORES)),
        trace=trace, trace_cores=trace_cores)


def kernel(x, w_qkv, pos_table, w_out):
    in_maps = host_prep(x, w_qkv, pos_table, w_out)
    res = run(in_maps)
    out = np.stack([r["out"] for r in res.results])    # [8, 2, 784, 512]
    return np.ascontiguousarray(out.reshape(B, HH, WW, C)).astype(np.float32)


# revision 13
# speedup vs baseline: 1.4213x; 1.0110x over previous
"""Trainium2 Bass kernel V3: MHSA with multi-head relative position embedding.

Data-parallel over batch: 16 batches / 8 cores = 2 per core, all 8 heads local.

V3: one flat software pipeline over 112 (head, k-tile) slots so the PE stream
never breaks (TRN2 p-states make every PE idle gap cost ~3us of half-clock).
Per slot: scores matmuls for slot j, attn@v matmuls for slot j-LAG (the trail
crosses head boundaries), plus column-budget-paced filler work (qkv projection
tiles for future pairs, out-projection once a batch finishes). The attn@v
accumulator is freed by ONE full-tile ACT copy (same ACT cost as copying just
the sumexp row, since ACT time only depends on the free dim), so op needs a
single 2-bank PSUM buffer and the normalize chain (DVE approx-recip, GPSIMD
partition-broadcast, DVE mult) runs entirely off PSUM/critical path.

PSUM: 2x scores [112,784] (2 banks each) + 1x op [65,784] (2 banks) +
2x 1-bank filler = 8 banks.
"""

import numpy as np
import ml_dtypes

B, HH, WW, C = 16, 28, 28, 512
N = HH * WW             # 784 tokens
HEADS, KD = 8, 64
NCORES, BPC = 8, 2      # 8 cores, 2 batches per core
NT, TP = 7, 112         # 784 = 7 k/token tiles of 112
CHUNKS = [(0, 512), (512, 272)]   # PSUM bank split of the 784-wide free dim
CT = 4                  # contraction tiles of 128 over C=512
NPAIRS = 4
LAG = 3                 # attn@v trails scores by this many slots

_CACHE = {}


def _rel_index():
    t = np.arange(N)
    c0, c1 = t % HH, t // HH
    return ((c0[:, None] - c0[None, :] + HH - 1)
            + (c1[:, None] - c1[None, :] + WW - 1) * (2 * HH - 1))  # [q, k]


def build_nc():
    if 'nc' in _CACHE:
        return _CACHE['nc']
    from contextlib import ExitStack
    import concourse.bacc as bacc
    import concourse.mybir as mybir
    import concourse.tile as tile
    from concourse.alu_op_type import AluOpType

    f32 = mybir.dt.float32
    bf16 = mybir.dt.bfloat16
    EXP = mybir.ActivationFunctionType.Exp

    nc = bacc.Bacc("TRN2", debug=False, enable_asserts=False)
    xT_d = nc.dram_tensor("xT", [BPC, C, N], bf16, kind="ExternalInput").ap()
    wqkv_d = nc.dram_tensor("wqkv", [C, 3 * C], bf16, kind="ExternalInput").ap()
    wout_d = nc.dram_tensor("wout", [C, C], bf16, kind="ExternalInput").ap()
    bias_d = nc.dram_tensor("biasT", [HEADS, N, N], bf16, kind="ExternalInput").ap()
    out_d = nc.dram_tensor("out", [BPC, N, C], f32, kind="ExternalOutput").ap()

    with tile.TileContext(nc) as tc, ExitStack() as ctx:
        persist = ctx.enter_context(tc.tile_pool(name="persist", bufs=1))
        bias_pool = ctx.enter_context(tc.tile_pool(name="biasp", bufs=28))
        eraw_pool = ctx.enter_context(tc.tile_pool(name="erp", bufs=4))
        esb_pool = ctx.enter_context(tc.tile_pool(name="ep", bufs=8))
        oc_pool = ctx.enter_context(tc.tile_pool(name="ocp", bufs=3))
        r_pool = ctx.enter_context(tc.tile_pool(name="rp", bufs=4))
        osb_pool = ctx.enter_context(tc.tile_pool(name="osbp", bufs=4))
        sc_psum = ctx.enter_context(tc.tile_pool(name="scp", bufs=2, space="PSUM"))
        op_psum = ctx.enter_context(tc.tile_pool(name="opp", bufs=1, space="PSUM"))
        fl_psum = ctx.enter_context(tc.tile_pool(name="flp", bufs=2, space="PSUM"))

        # ---- persistent SBUF tensors ----
        wqkv_sb = [persist.tile([128, 3 * C], bf16, tag=f"wqkv{ci}",
                                name=f"wqkv{ci}") for ci in range(CT)]
        wout_sb = [persist.tile([128, C], bf16, tag=f"wout{ci}",
                                name=f"woutw{ci}") for ci in range(CT)]
        xts = {(b, ci): persist.tile([128, N], bf16, tag=f"xT{b}_{ci}",
                                     name=f"xT{b}_{ci}")
               for b in range(BPC) for ci in range(CT)}
        qkT = {(b, ft): persist.tile([128, N], bf16, tag=f"qkT{b}_{ft}",
                                     name=f"qkT{b}_{ft}")
               for b in range(BPC) for ft in range(8)}
        vsb = {(b, t): persist.tile([TP, HEADS, KD + 1], bf16, tag=f"v{b}_{t}",
                                    name=f"v{b}_{t}")
               for b in range(BPC) for t in range(NT)}
        attnT = {(b, fi): persist.tile([128, N], bf16, tag=f"attnT{b}_{fi}",
                                       name=f"attnT{b}_{fi}")
                 for b in range(BPC) for fi in range(CT)}

        # ---- input DMAs on two issue queues (sync + gpsimd) so the DGE
        # config time (~600ns per dma_start) does not serialize the start ----
        for ci in range(CT):
            nc.sync.dma_start(xts[0, ci], xT_d[0, ci * 128:(ci + 1) * 128, :])
            nc.gpsimd.dma_start(wqkv_sb[ci], wqkv_d[ci * 128:(ci + 1) * 128, :])
        for ci in range(CT):
            nc.sync.dma_start(xts[1, ci], xT_d[1, ci * 128:(ci + 1) * 128, :])
            nc.gpsimd.dma_start(wout_sb[ci], wout_d[ci * 128:(ci + 1) * 128, :])

        bias_sb = {}

        def fetch_bias_tile(h, kt, queue):
            bt = bias_pool.tile([TP, N], bf16, tag="bias", name=f"bias{h}_{kt}")
            queue.dma_start(bt, bias_d[h, kt * TP:(kt + 1) * TP, :])
            bias_sb[h, kt] = bt

        def fetch_bias(pair, queue):
            for h in (2 * pair, 2 * pair + 1):
                for kt in range(NT):
                    fetch_bias_tile(h, kt, queue)

        fetch_bias(0, nc.sync)
        fetch_bias(1, nc.sync)

        # ---- filler work units; each returns its PE column cost ----
        def emit_qk_chunk(b, ft, c0w, cw, on_act=False):
            ps = fl_psum.tile([128, 512], f32, tag="fl", name=f"pj{b}_{ft}_{c0w}")
            for ci in range(CT):
                nc.tensor.matmul(
                    ps[:, 0:cw], wqkv_sb[ci][:, ft * 128:(ft + 1) * 128],
                    xts[b, ci][:, c0w:c0w + cw],
                    start=(ci == 0), stop=(ci == CT - 1))
            eng = nc.scalar.copy if on_act else nc.vector.tensor_copy
            eng(qkT[b, ft][:, c0w:c0w + cw], ps[:, 0:cw])
            return CT * cw

        def emit_v_tile(b, t, pair, on_act=False):
            ps = fl_psum.tile([128, 512], f32, tag="fl", name=f"pv{b}_{t}_{pair}")
            f0 = 2 * C + 128 * pair
            for ci in range(CT):
                nc.tensor.matmul(
                    ps[0:TP, 0:128], xts[b, ci][:, t * TP:(t + 1) * TP],
                    wqkv_sb[ci][:, f0:f0 + 128],
                    start=(ci == 0), stop=(ci == CT - 1))
            eng = nc.scalar.copy if on_act else nc.vector.tensor_copy
            eng(vsb[b, t][:, 2 * pair:2 * pair + 2, 0:KD],
                ps[0:TP, 0:128].rearrange("p (h d) -> p h d", h=2))
            if pair == 0:
                nc.vector.memset(vsb[b, t][:, :, KD:KD + 1], 1.0)
            return CT * 128

        def emit_oproj_prefix(b, t):
            ps = fl_psum.tile([128, 512], f32, tag="fl", name=f"po{b}_{t}")
            for fi in range(CT - 1):
                nc.tensor.matmul(
                    ps[0:TP, :], attnT[b, fi][:, t * TP:(t + 1) * TP], wout_sb[fi],
                    start=(fi == 0), stop=False)
            return ps

        def emit_oproj_finish(b, t, ps, on_act, split_dma=False):
            fi = CT - 1
            nc.tensor.matmul(
                ps[0:TP, :], attnT[b, fi][:, t * TP:(t + 1) * TP], wout_sb[fi],
                start=False, stop=True)
            osb = osb_pool.tile([TP, C], f32, tag="osb", name="osb")
            eng = nc.scalar.copy if on_act else nc.vector.tensor_copy
            if split_dma:
                eng(osb[:, 0:C // 2], ps[0:TP, 0:C // 2])
                nc.sync.dma_start(out_d[b, t * TP:(t + 1) * TP, 0:C // 2],
                                  osb[:, 0:C // 2])
                eng(osb[:, C // 2:C], ps[0:TP, C // 2:C])
                nc.sync.dma_start(out_d[b, t * TP:(t + 1) * TP, C // 2:C],
                                  osb[:, C // 2:C])
            else:
                eng(osb, ps[0:TP, :])
                nc.sync.dma_start(out_d[b, t * TP:(t + 1) * TP, :], osb)

        def emit_oproj_tile(b, t, on_act):
            ps = emit_oproj_prefix(b, t)
            emit_oproj_finish(b, t, ps, on_act)
            return CT * C

        # ---- flat slot pipeline ----
        slots = [(pair, b, hh, kt)
                 for pair in range(NPAIRS)
                 for b in range(BPC)
                 for hh in range(2)
                 for kt in range(NT)]
        nslots = len(slots)

        esbs = {}        # (head-id, kt) -> esb tile
        ops = {}         # head-id -> op psum tile

        def emit_scores(pair, b, h, kt):
            hh = h % 2
            r0 = hh * 64
            sc = sc_psum.tile([TP, N], f32, tag="sc", name=f"sc{b}_{h}_{kt}")
            for (c0w, cw) in CHUNKS:
                nc.tensor.matmul(
                    sc[:, c0w:c0w + cw],
                    qkT[b, 4 + pair][r0:r0 + 64, kt * TP:(kt + 1) * TP],
                    qkT[b, pair][r0:r0 + 64, c0w:c0w + cw],
                    start=True, stop=True)
            eraw = eraw_pool.tile([TP, N], bf16, tag="eraw",
                                  name=f"er{b}_{h}_{kt}")
            nc.scalar.activation(eraw, sc, EXP)
            esb = esb_pool.tile([TP, N], bf16, tag="e", name=f"e{b}_{h}_{kt}")
            nc.vector.tensor_tensor(esb, eraw, bias_sb[h, kt], AluOpType.mult)
            esbs[(pair, b, h), kt] = esb

        def emit_attnv(pair, b, h, kt):
            hid = (pair, b, h)
            if kt == 0:
                ops[hid] = op_psum.tile([KD + 1, N], f32, tag="op",
                                        name=f"op{b}_{h}")
            op = ops[hid]
            for (c0w, cw) in CHUNKS:
                nc.tensor.matmul(
                    op[:, c0w:c0w + cw], vsb[b, kt][:, h:h + 1, :],
                    esbs[hid, kt][:, c0w:c0w + cw],
                    start=(kt == 0), stop=(kt == NT - 1))

        def emit_normalize(pair, b, h):
            hh = h % 2
            r0 = hh * 64
            op = ops.pop((pair, b, h))
            # two ACT copies free the op psum fast; the custom-DVE recip needs
            # its input at partition 0 and the value rows copy with an even
            # partition count (the fused [65,784] variant corrupts on HW)
            srow = r_pool.tile([1, N], f32, tag="srow", name="srow")
            nc.scalar.copy(srow, op[KD:KD + 1, :])
            rrow = r_pool.tile([1, N], f32, tag="rrow", name="rrow")
            nc.vector.reciprocal_approx_fast(rrow, srow)
            oc = oc_pool.tile([KD, N], f32, tag="oc", name=f"oc{b}_{h}")
            # HW: every ACT instr costs a flat ~914ns while DVE copies are
            # ~2x cheaper -- keep ACT exp-only (srow stays on ACT: only ACT
            # handles the cross-partition sumexp-row copy correctly)
            nc.vector.tensor_copy(oc, op[0:KD, :])
            rb = r_pool.tile([64, N], f32, tag="rb", name="rb")
            nc.gpsimd.partition_broadcast(rb, rrow)
            nc.vector.tensor_tensor(
                attnT[b, pair][r0:r0 + 64, :], oc, rb, AluOpType.mult)

        # warmup: projections for pair 0 as one straight PE stream
        with nc.named_scope("warmup"):
            i = 0
            for b in range(BPC):
                for ft in (0, 4):
                    for (c0w, cw) in CHUNKS:
                        emit_qk_chunk(b, ft, c0w, cw, on_act=(i % 2 == 0))
                        i += 1
            for b in range(BPC):
                for t in range(NT):
                    emit_v_tile(b, t, 0)

        # static filler schedule: slot -> [unit, ...]
        sched = {}

        def sched_add(j, fn):
            sched.setdefault(j, []).append(fn)

        for pair in range(1, NPAIRS):
            base = 28 * (pair - 1)
            for i, (b, ft) in enumerate(
                    (b, ft) for b in range(BPC) for ft in (pair, 4 + pair)):
                for k, (c0w, cw) in enumerate(CHUNKS):
                    sched_add(base + 1 + 3 * (2 * i + k),
                              lambda b=b, ft=ft, c0w=c0w, cw=cw:
                              emit_qk_chunk(b, ft, c0w, cw))
            for b in range(BPC):
                for t in range(NT):
                    sched_add(max(0, 28 * pair + 14 * b + t - 3),
                              lambda b=b, t=t, pair=pair:
                              emit_v_tile(b, t, pair))

        oproj_q = []
        bias_fetch_q = []
        for j in range(nslots + LAG):
            for _ in range(2):
                if bias_fetch_q:
                    h, k = bias_fetch_q.pop(0)
                    fetch_bias_tile(h, k, nc.sync)
            # scores for slot j
            if j < nslots:
                pair, b, hh, kt = slots[j]
                if hh == 0 and b == 0 and kt == 0 and pair + 2 < NPAIRS:
                    bias_fetch_q.extend(
                        (h, k) for h in (2 * pair + 4, 2 * pair + 5)
                        for k in range(NT))
                emit_scores(pair, b, 2 * pair + hh, kt)
            # attn@v for slot j-LAG, then normalize at head end
            jj = j - LAG
            if jj >= 0:
                pair, b, hh, kt = slots[jj]
                h = 2 * pair + hh
                emit_attnv(pair, b, h, kt)
                if kt == NT - 1:
                    emit_normalize(pair, b, h)
                    if pair == NPAIRS - 1 and hh == 1 and b == 0:
                        oproj_q.extend(
                            lambda t=t: emit_oproj_tile(0, t, on_act=False)
                            for t in range(NT))
            for fn in sched.pop(j, ()):
                fn()
            if oproj_q:
                oproj_q.pop(0)()

        with nc.named_scope("tail"):
            # last batch's out-projection: emit the fi<3 accumulations first
            # (their attnT inputs finished pairs ago) so the PE streams while
            # the final head's normalize chain completes.
            pend = []
            for t in range(NT):
                if len(pend) < 2:
                    pend.append((t, emit_oproj_prefix(1, t)))
            nxt = len(pend)
            while pend:
                t, ps = pend.pop(0)
                emit_oproj_finish(1, t, ps, on_act=True,
                                  split_dma=(t == NT - 1))
                if nxt < NT:
                    pend.append((nxt, emit_oproj_prefix(1, nxt)))
                    nxt += 1

    nc.compile()
    _CACHE['nc'] = nc
    return nc


def host_prep(x, w_qkv, pos_table, w_out):
    x = np.asarray(x, np.float32).reshape(B, N, C)
    wq = np.array(np.asarray(w_qkv, np.float32), copy=True)
    wq[:, :C] *= np.float32(1.0 / np.sqrt(KD))
    wq_bf = wq.astype(ml_dtypes.bfloat16)
    idx = _rel_index()
    biasT = np.ascontiguousarray(np.exp(
        np.asarray(pos_table, np.float32)[:, idx].transpose(0, 2, 1)
    )).astype(ml_dtypes.bfloat16)
    wout = np.ascontiguousarray(np.asarray(w_out, np.float32)).astype(
        ml_dtypes.bfloat16)
    in_maps = []
    for c in range(NCORES):
        xT = np.ascontiguousarray(
            x[c * BPC:(c + 1) * BPC].transpose(0, 2, 1)).astype(
                ml_dtypes.bfloat16)  # [2, 512, 784]
        in_maps.append({"xT": xT, "wqkv": wq_bf, "wout": wout, "biasT": biasT})
    return in_maps


def run(in_maps, trace=False, trace_cores=None):
    import concourse.bass_utils as bass_utils
    nc = build_nc()
    return bass_utils.run_bass_kernel_spmd(
        nc, in_maps, core_ids=list(range(NCORES)),
        trace=trace, trace_cores=trace_cores)


def kernel(x, w_qkv, pos_table, w_out):
    in_maps = host_prep(x, w_qkv, pos_table, w_out)
    res = run(in_maps)
    out = np.stack([r["out"] for r in res.results])    # [8, 2, 784, 512]
    return np.ascontiguousarray(out.reshape(B, HH, WW, C)).astype(np.float32)


# revision 14
# speedup vs baseline: 1.4218x; 1.0004x over previous
"""Trainium2 Bass kernel V3: MHSA with multi-head relative position embedding.

Data-parallel over batch: 16 batches / 8 cores = 2 per core, all 8 heads local.

V3: one flat software pipeline over 112 (head, k-tile) slots so the PE stream
never breaks (TRN2 p-states make every PE idle gap cost ~3us of half-clock).
Per slot: scores matmuls for slot j, attn@v matmuls for slot j-LAG (the trail
crosses head boundaries), plus column-budget-paced filler work (qkv projection
tiles for future pairs, out-projection once a batch finishes). The attn@v
accumulator is freed by ONE full-tile ACT copy (same ACT cost as copying just
the sumexp row, since ACT time only depends on the free dim), so op needs a
single 2-bank PSUM buffer and the normalize chain (DVE approx-recip, GPSIMD
partition-broadcast, DVE mult) runs entirely off PSUM/critical path.

PSUM: 2x scores [112,784] (2 banks each) + 1x op [65,784] (2 banks) +
2x 1-bank filler = 8 banks.
"""

import numpy as np
import ml_dtypes

B, HH, WW, C = 16, 28, 28, 512
N = HH * WW             # 784 tokens
HEADS, KD = 8, 64
NCORES, BPC = 8, 2      # 8 cores, 2 batches per core
NT, TP = 7, 112         # 784 = 7 k/token tiles of 112
CHUNKS = [(0, 512), (512, 272)]   # PSUM bank split of the 784-wide free dim
CT = 4                  # contraction tiles of 128 over C=512
NPAIRS = 4
LAG = 3                 # attn@v trails scores by this many slots

_CACHE = {}


def _rel_index():
    t = np.arange(N)
    c0, c1 = t % HH, t // HH
    return ((c0[:, None] - c0[None, :] + HH - 1)
            + (c1[:, None] - c1[None, :] + WW - 1) * (2 * HH - 1))  # [q, k]


def build_nc():
    if 'nc' in _CACHE:
        return _CACHE['nc']
    from contextlib import ExitStack
    import concourse.bacc as bacc
    import concourse.mybir as mybir
    import concourse.tile as tile
    from concourse.alu_op_type import AluOpType

    f32 = mybir.dt.float32
    bf16 = mybir.dt.bfloat16
    EXP = mybir.ActivationFunctionType.Exp

    nc = bacc.Bacc("TRN2", debug=False, enable_asserts=False)
    xT_d = nc.dram_tensor("xT", [BPC, C, N], bf16, kind="ExternalInput").ap()
    wqkv_d = nc.dram_tensor("wqkv", [C, 3 * C], bf16, kind="ExternalInput").ap()
    wout_d = nc.dram_tensor("wout", [C, C], bf16, kind="ExternalInput").ap()
    bias_d = nc.dram_tensor("biasT", [HEADS, N, N], bf16, kind="ExternalInput").ap()
    out_d = nc.dram_tensor("out", [BPC, N, C], f32, kind="ExternalOutput").ap()

    with tile.TileContext(nc) as tc, ExitStack() as ctx:
        persist = ctx.enter_context(tc.tile_pool(name="persist", bufs=1))
        bias_pool = ctx.enter_context(tc.tile_pool(name="biasp", bufs=28))
        eraw_pool = ctx.enter_context(tc.tile_pool(name="erp", bufs=4))
        esb_pool = ctx.enter_context(tc.tile_pool(name="ep", bufs=8))
        oc_pool = ctx.enter_context(tc.tile_pool(name="ocp", bufs=3))
        r_pool = ctx.enter_context(tc.tile_pool(name="rp", bufs=4))
        osb_pool = ctx.enter_context(tc.tile_pool(name="osbp", bufs=4))
        sc_psum = ctx.enter_context(tc.tile_pool(name="scp", bufs=2, space="PSUM"))
        op_psum = ctx.enter_context(tc.tile_pool(name="opp", bufs=1, space="PSUM"))
        fl_psum = ctx.enter_context(tc.tile_pool(name="flp", bufs=2, space="PSUM"))

        # ---- persistent SBUF tensors ----
        wqkv_sb = [persist.tile([128, 3 * C], bf16, tag=f"wqkv{ci}",
                                name=f"wqkv{ci}") for ci in range(CT)]
        wout_sb = [persist.tile([128, C], bf16, tag=f"wout{ci}",
                                name=f"woutw{ci}") for ci in range(CT)]
        xts = {(b, ci): persist.tile([128, N], bf16, tag=f"xT{b}_{ci}",
                                     name=f"xT{b}_{ci}")
               for b in range(BPC) for ci in range(CT)}
        qkT = {(b, ft): persist.tile([128, N], bf16, tag=f"qkT{b}_{ft}",
                                     name=f"qkT{b}_{ft}")
               for b in range(BPC) for ft in range(8)}
        vsb = {(b, t): persist.tile([TP, HEADS, KD + 1], bf16, tag=f"v{b}_{t}",
                                    name=f"v{b}_{t}")
               for b in range(BPC) for t in range(NT)}
        attnT = {(b, fi): persist.tile([128, N], bf16, tag=f"attnT{b}_{fi}",
                                       name=f"attnT{b}_{fi}")
                 for b in range(BPC) for fi in range(CT)}

        # ---- input DMAs on two issue queues (sync + gpsimd) so the DGE
        # config time (~600ns per dma_start) does not serialize the start ----
        for ci in range(CT):
            nc.sync.dma_start(xts[0, ci], xT_d[0, ci * 128:(ci + 1) * 128, :])
            nc.gpsimd.dma_start(wqkv_sb[ci], wqkv_d[ci * 128:(ci + 1) * 128, :])
        for ci in range(CT):
            nc.sync.dma_start(xts[1, ci], xT_d[1, ci * 128:(ci + 1) * 128, :])
            nc.gpsimd.dma_start(wout_sb[ci], wout_d[ci * 128:(ci + 1) * 128, :])

        bias_sb = {}

        def fetch_bias_tile(h, kt, queue):
            bt = bias_pool.tile([TP, N], bf16, tag="bias", name=f"bias{h}_{kt}")
            queue.dma_start(bt, bias_d[h, kt * TP:(kt + 1) * TP, :])
            bias_sb[h, kt] = bt

        def fetch_bias(pair, queue):
            for h in (2 * pair, 2 * pair + 1):
                for kt in range(NT):
                    fetch_bias_tile(h, kt, queue)

        fetch_bias(0, nc.sync)
        fetch_bias(1, nc.sync)

        # ---- filler work units; each returns its PE column cost ----
        def emit_qk_chunk(b, ft, c0w, cw, on_act=False):
            ps = fl_psum.tile([128, 512], f32, tag="fl", name=f"pj{b}_{ft}_{c0w}")
            for ci in range(CT):
                nc.tensor.matmul(
                    ps[:, 0:cw], wqkv_sb[ci][:, ft * 128:(ft + 1) * 128],
                    xts[b, ci][:, c0w:c0w + cw],
                    start=(ci == 0), stop=(ci == CT - 1))
            eng = nc.scalar.copy if on_act else nc.vector.tensor_copy
            eng(qkT[b, ft][:, c0w:c0w + cw], ps[:, 0:cw])
            return CT * cw

        def emit_v_tile(b, t, pair, on_act=False):
            ps = fl_psum.tile([128, 512], f32, tag="fl", name=f"pv{b}_{t}_{pair}")
            f0 = 2 * C + 128 * pair
            for ci in range(CT):
                nc.tensor.matmul(
                    ps[0:TP, 0:128], xts[b, ci][:, t * TP:(t + 1) * TP],
                    wqkv_sb[ci][:, f0:f0 + 128],
                    start=(ci == 0), stop=(ci == CT - 1))
            eng = nc.scalar.copy if on_act else nc.vector.tensor_copy
            eng(vsb[b, t][:, 2 * pair:2 * pair + 2, 0:KD],
                ps[0:TP, 0:128].rearrange("p (h d) -> p h d", h=2))
            if pair == 0:
                nc.vector.memset(vsb[b, t][:, :, KD:KD + 1], 1.0)
            return CT * 128

        def emit_oproj_prefix(b, t):
            ps = fl_psum.tile([128, 512], f32, tag="fl", name=f"po{b}_{t}")
            for fi in range(CT - 1):
                nc.tensor.matmul(
                    ps[0:TP, :], attnT[b, fi][:, t * TP:(t + 1) * TP], wout_sb[fi],
                    start=(fi == 0), stop=False)
            return ps

        def emit_oproj_finish(b, t, ps, on_act, split_dma=False):
            fi = CT - 1
            nc.tensor.matmul(
                ps[0:TP, :], attnT[b, fi][:, t * TP:(t + 1) * TP], wout_sb[fi],
                start=False, stop=True)
            osb = osb_pool.tile([TP, C], f32, tag="osb", name="osb")
            eng = nc.scalar.copy if on_act else nc.vector.tensor_copy
            if split_dma:
                eng(osb[:, 0:C // 2], ps[0:TP, 0:C // 2])
                nc.sync.dma_start(out_d[b, t * TP:(t + 1) * TP, 0:C // 2],
                                  osb[:, 0:C // 2])
                eng(osb[:, C // 2:C], ps[0:TP, C // 2:C])
                nc.sync.dma_start(out_d[b, t * TP:(t + 1) * TP, C // 2:C],
                                  osb[:, C // 2:C])
            else:
                eng(osb, ps[0:TP, :])
                nc.sync.dma_start(out_d[b, t * TP:(t + 1) * TP, :], osb)

        def emit_oproj_tile(b, t, on_act):
            ps = emit_oproj_prefix(b, t)
            emit_oproj_finish(b, t, ps, on_act)
            return CT * C

        # ---- flat slot pipeline ----
        slots = [(pair, b, hh, kt)
                 for pair in range(NPAIRS)
                 for b in range(BPC)
                 for hh in range(2)
                 for kt in range(NT)]
        nslots = len(slots)

        esbs = {}        # (head-id, kt) -> esb tile
        ops = {}         # head-id -> op psum tile

        def emit_scores(pair, b, h, kt):
            hh = h % 2
            r0 = hh * 64
            sc = sc_psum.tile([TP, N], f32, tag="sc", name=f"sc{b}_{h}_{kt}")
            for (c0w, cw) in CHUNKS:
                nc.tensor.matmul(
                    sc[:, c0w:c0w + cw],
                    qkT[b, 4 + pair][r0:r0 + 64, kt * TP:(kt + 1) * TP],
                    qkT[b, pair][r0:r0 + 64, c0w:c0w + cw],
                    start=True, stop=True)
            eraw = eraw_pool.tile([TP, N], bf16, tag="eraw",
                                  name=f"er{b}_{h}_{kt}")
            nc.scalar.activation(eraw, sc, EXP)
            esb = esb_pool.tile([TP, N], bf16, tag="e", name=f"e{b}_{h}_{kt}")
            nc.vector.tensor_tensor(esb, eraw, bias_sb[h, kt], AluOpType.mult)
            esbs[(pair, b, h), kt] = esb

        def emit_attnv(pair, b, h, kt):
            hid = (pair, b, h)
            if kt == 0:
                ops[hid] = op_psum.tile([KD + 1, N], f32, tag="op",
                                        name=f"op{b}_{h}")
            op = ops[hid]
            for (c0w, cw) in CHUNKS:
                nc.tensor.matmul(
                    op[:, c0w:c0w + cw], vsb[b, kt][:, h:h + 1, :],
                    esbs[hid, kt][:, c0w:c0w + cw],
                    start=(kt == 0), stop=(kt == NT - 1))

        def emit_normalize(pair, b, h):
            hh = h % 2
            r0 = hh * 64
            op = ops.pop((pair, b, h))
            # two ACT copies free the op psum fast; the custom-DVE recip needs
            # its input at partition 0 and the value rows copy with an even
            # partition count (the fused [65,784] variant corrupts on HW)
            srow = r_pool.tile([1, N], f32, tag="srow", name="srow")
            nc.scalar.copy(srow, op[KD:KD + 1, :])
            rrow = r_pool.tile([1, N], f32, tag="rrow", name="rrow")
            nc.vector.reciprocal_approx_fast(rrow, srow)
            oc = oc_pool.tile([KD, N], f32, tag="oc", name=f"oc{b}_{h}")
            # HW: every ACT instr costs a flat ~914ns while DVE copies are
            # ~2x cheaper -- keep ACT exp-only (srow stays on ACT: only ACT
            # handles the cross-partition sumexp-row copy correctly)
            nc.vector.tensor_copy(oc, op[0:KD, :])
            rb = r_pool.tile([64, N], f32, tag="rb", name="rb")
            nc.gpsimd.partition_broadcast(rb, rrow)
            nc.vector.tensor_tensor(
                attnT[b, pair][r0:r0 + 64, :], oc, rb, AluOpType.mult)

        # warmup: projections for pair 0 as one straight PE stream
        with nc.named_scope("warmup"):
            i = 0
            for b in range(BPC):
                for ft in (0, 4):
                    for (c0w, cw) in CHUNKS:
                        emit_qk_chunk(b, ft, c0w, cw, on_act=(i % 2 == 0))
                        i += 1
            for b in range(BPC):
                for t in range(NT):
                    emit_v_tile(b, t, 0)

        # static filler schedule: slot -> [unit, ...]
        sched = {}

        def sched_add(j, fn):
            sched.setdefault(j, []).append(fn)

        for pair in range(1, NPAIRS):
            base = 28 * (pair - 1)
            for i, (b, ft) in enumerate(
                    (b, ft) for b in range(BPC) for ft in (pair, 4 + pair)):
                for k, (c0w, cw) in enumerate(CHUNKS):
                    sched_add(base + 1 + 3 * (2 * i + k),
                              lambda b=b, ft=ft, c0w=c0w, cw=cw:
                              emit_qk_chunk(b, ft, c0w, cw))
            for b in range(BPC):
                for t in range(NT):
                    sched_add(max(0, 28 * pair + 14 * b + t - 3),
                              lambda b=b, t=t, pair=pair:
                              emit_v_tile(b, t, pair))

        oproj_q = []
        bias_fetch_q = []
        for j in range(nslots + LAG):
            for _ in range(2):
                if bias_fetch_q:
                    h, k = bias_fetch_q.pop(0)
                    fetch_bias_tile(h, k, nc.sync)
            # scores for slot j
            if j < nslots:
                pair, b, hh, kt = slots[j]
                if hh == 0 and b == 0 and kt == 0 and pair + 2 < NPAIRS:
                    bias_fetch_q.extend(
                        (h, k) for h in (2 * pair + 4, 2 * pair + 5)
                        for k in range(NT))
                emit_scores(pair, b, 2 * pair + hh, kt)
            # attn@v for slot j-LAG, then normalize at head end
            jj = j - LAG
            if jj >= 0:
                pair, b, hh, kt = slots[jj]
                h = 2 * pair + hh
                emit_attnv(pair, b, h, kt)
                if kt == NT - 1:
                    emit_normalize(pair, b, h)
                    if pair == NPAIRS - 1 and hh == 1 and b == 0:
                        oproj_q.extend(
                            lambda t=t: emit_oproj_tile(0, t, on_act=False)
                            for t in range(NT))
            for fn in sched.pop(j, ()):
                fn()
            if oproj_q and j % 2 == 0:
                oproj_q.pop(0)()

        with nc.named_scope("tail"):
            # last batch's out-projection: emit the fi<3 accumulations first
            # (their attnT inputs finished pairs ago) so the PE streams while
            # the final head's normalize chain completes.
            pend = []
            for t in range(NT):
                if len(pend) < 2:
                    pend.append((t, emit_oproj_prefix(1, t)))
            nxt = len(pend)
            while pend:
                t, ps = pend.pop(0)
                emit_oproj_finish(1, t, ps, on_act=True,
                                  split_dma=(t == NT - 1))
                if nxt < NT:
                    pend.append((nxt, emit_oproj_prefix(1, nxt)))
                    nxt += 1

    nc.compile()
    _CACHE['nc'] = nc
    return nc


def host_prep(x, w_qkv, pos_table, w_out):
    x = np.asarray(x, np.float32).reshape(B, N, C)
    wq = np.array(np.asarray(w_qkv, np.float32), copy=True)
    wq[:, :C] *= np.float32(1.0 / np.sqrt(KD))
    wq_bf = wq.astype(ml_dtypes.bfloat16)
    idx = _rel_index()
    biasT = np.ascontiguousarray(np.exp(
        np.asarray(pos_table, np.float32)[:, idx].transpose(0, 2, 1)
    )).astype(ml_dtypes.bfloat16)
    wout = np.ascontiguousarray(np.asarray(w_out, np.float32)).astype(
        ml_dtypes.bfloat16)
    in_maps = []
    for c in range(NCORES):
        xT = np.ascontiguousarray(
            x[c * BPC:(c + 1) * BPC].transpose(0, 2, 1)).astype(
                ml_dtypes.bfloat16)  # [2, 512, 784]
        in_maps.append({"xT": xT, "wqkv": wq_bf, "wout": wout, "biasT": biasT})
    return in_maps


def run(in_maps, trace=False, trace_cores=None):
    import concourse.bass_utils as bass_utils
    nc = build_nc()
    return bass_utils.run_bass_kernel_spmd(
        nc, in_maps, core_ids=list(range(NCORES)),
        trace=trace, trace_cores=trace_cores)


def kernel(x, w_qkv, pos_table, w_out):
    in_maps = host_prep(x, w_qkv, pos_table, w_out)
    res = run(in_maps)
    out = np.stack([r["out"] for r in res.results])    # [8, 2, 784, 512]
    return np.ascontiguousarray(out.reshape(B, HH, WW, C)).astype(np.float32)
